# revision 22
# baseline (speedup 1.0000x reference)
"""Expert-parallel top-1 MoE FFN kernel for 8 Trainium2 NeuronCores.

Problem: x[2,2048,1024] routed (top-1 of softmax(x @ rW.T + rb)) through one of
E=8 expert FFNs (Linear(1024,4096) -> gelu -> Linear(4096,1024)).
Returns (expert_outputs, router_probs, router_logits).

Strategy (one expert per core):
  - router is data-parallel: core c computes logits/probs/one-hot for tokens
    [512c, 512c+512) (PE-transposed x slice, exact fp32 matmul)
  - one-hot masks are AllGather'd (4096x8 fp32, tiny)
  - each core compacts the token ids routed to ITS expert with an on-device
    prefix-sum (tensor_tensor_scan + triangular matmul) and an indirect-DMA
    scatter into a DRAM perm table (pad slots -> OOB, silently skipped)
  - indirect-DMA gather of those <=C=640 token rows, FFN with float32r
    matmuls arranged so W1/W2 stream from HBM exactly once,
    indirect-DMA scatter of the C output rows back to the full output
  - host sums the 8 disjoint per-core outputs and concatenates router slices
"""

import numpy as np

import concourse.bass as bass
import concourse.tile as tile
from concourse import bacc, mybir
from concourse.bass import IndirectOffsetOnAxis
from concourse.masks import make_identity, make_upper_triangular

F32 = mybir.dt.float32
F32R = mybir.dt.float32r
I32 = mybir.dt.int32
AX = mybir.AxisListType
OP = mybir.AluOpType
ACT = mybir.ActivationFunctionType

P = 128
NTOK = 4096
D = 1024
F = 4096
E = 8
NCORES = 8
SLICE = NTOK // NCORES          # router tokens per core
C = 640                         # expert token capacity per core
CHUNKS = [(0, 0, 384), (1, 384, 256)]  # (psum bank, slot offset, width); >=256 wide
                                        # so float32r runs at full rate
NI = NTOK // P                  # 32 token-tiles (t = p + 128*i)
ND = D // P                     # 8
NF = F // P                     # 32
NST = C // P                    # 5 slot tiles
OOB_PAD = 100000.0              # slot value for tokens not on this expert


def build_nc(act_fn=None):
    nc = bacc.Bacc("TRN2", target_bir_lowering=False, debug=False,
                   num_devices=NCORES)

    x = nc.dram_tensor("x", [NTOK, D], F32, kind="ExternalInput").ap()
    xsl = nc.dram_tensor("xsl", [SLICE, D], F32, kind="ExternalInput").ap()
    rwt = nc.dram_tensor("rwt", [D, E], F32, kind="ExternalInput").ap()
    rb = nc.dram_tensor("rb", [1, E], F32, kind="ExternalInput").ap()
    w1t = nc.dram_tensor("w1t", [ND, NF, P, P], F32, kind="ExternalInput").ap()
    b1 = nc.dram_tensor("b1", [F, 1], F32, kind="ExternalInput").ap()
    w2t = nc.dram_tensor("w2t", [ND, NF, P, P], F32, kind="ExternalInput").ap()
    b2 = nc.dram_tensor("b2", [D, 1], F32, kind="ExternalInput").ap()
    sel = nc.dram_tensor("sel", [1, 1, E], F32, kind="ExternalInput").ap()

    out = nc.dram_tensor("out", [NTOK, D], F32, kind="ExternalOutput").ap()
    lgout = nc.dram_tensor("lgout", [SLICE, E], F32, kind="ExternalOutput").ap()
    prout = nc.dram_tensor("prout", [SLICE, E], F32, kind="ExternalOutput").ap()

    with tile.TileContext(nc) as tc:
        build_kernel(tc, x, xsl, rwt, rb, w1t, b1, w2t, b2, sel,
                     out, lgout, prout, act_fn=act_fn or ACT.Gelu)
    nc.compile()
    return nc


def build_kernel(tc, x, xsl, rwt, rb, w1t, b1, w2t, b2, sel,
                 out, lgout, prout, act_fn=ACT.Gelu):
    nc = tc.nc
    with (
        tc.tile_pool(name="const", bufs=1) as const,
        tc.tile_pool(name="xs", bufs=2) as xs_pool,
        tc.tile_pool(name="xT", bufs=1) as xT_pool,
        tc.tile_pool(name="small", bufs=4) as small,
        tc.tile_pool(name="routr", bufs=4) as routr,
        tc.tile_pool(name="xg", bufs=1) as xg_pool,
        tc.tile_pool(name="hT", bufs=1) as hT_pool,
        tc.tile_pool(name="wstream", bufs=8) as wstream,
        tc.tile_pool(name="yy", bufs=1) as y_pool,
        tc.tile_pool(name="psA", bufs=2, space="PSUM") as psA,      # [128,640] x2
        tc.tile_pool(name="psB", bufs=1, space="PSUM") as psB,      # [128,640]
        tc.tile_pool(name="psT", bufs=2, space="PSUM") as psT,      # [128,128] x2
        tc.tile_pool(name="dram", bufs=1, space="DRAM") as dram,
    ):
        # ---- constants ----
        ident = const.tile([P, P], F32)
        make_identity(nc, ident[:])
        ut = const.tile([P, P], F32)                  # strictly-upper ones
        make_upper_triangular(nc, ut[:], val=1.0, diag=False)
        rwt_sb = const.tile([P, ND, E], F32)
        nc.sync.dma_start(out=rwt_sb[:], in_=rwt.rearrange("(dt p) e -> p dt e", p=P))
        rb_sb = const.tile([P, E], F32)
        nc.sync.dma_start(out=rb_sb[:], in_=rb.to_broadcast([P, E]))
        sel_sb = const.tile([P, 1, E], F32)
        nc.sync.dma_start(out=sel_sb[:], in_=sel.to_broadcast([P, 1, E]))
        b1_sb = const.tile([P, NF], F32)
        nc.sync.dma_start(out=b1_sb[:], in_=b1.rearrange("(ft p) one -> p (ft one)", p=P))
        b2_sb = const.tile([P, ND], F32)
        nc.sync.dma_start(out=b2_sb[:], in_=b2.rearrange("(dt p) one -> p (dt one)", p=P))
        zeros32 = const.tile([P, NI], F32)
        nc.vector.memset(zeros32[:], 0.0)

        # ---- phase 1: router on own slice ----
        nsl = SLICE // P  # 4 token-tiles in the slice
        xT = xT_pool.tile([P, ND, SLICE], F32)
        ag_in = dram.tile([SLICE, E], F32)
        for tt in range(nsl):
            xs = xs_pool.tile([P, D], F32)
            nc.sync.dma_start(out=xs[:], in_=xsl[tt * P:(tt + 1) * P, :])
            for dt_ in range(ND):
                pst = psT.tile([P, P], F32, space="PSUM", tag="pst")
                nc.tensor.transpose(out=pst[:], in_=xs[:, dt_ * P:(dt_ + 1) * P],
                                    identity=ident[:])
                nc.vector.tensor_copy(out=xT[:, dt_, tt * P:(tt + 1) * P], in_=pst[:])
        for tt in range(nsl):
            psl = psT.tile([P, E], F32, space="PSUM", tag="pst")
            for dt_ in range(ND):
                nc.tensor.matmul(psl[:], lhsT=xT[:, dt_, tt * P:(tt + 1) * P],
                                 rhs=rwt_sb[:, dt_, :],
                                 start=(dt_ == 0), stop=(dt_ == ND - 1))
            lg = routr.tile([P, E], F32, tag="lg")
            nc.vector.tensor_tensor(out=lg[:], in0=psl[:], in1=rb_sb[:], op=OP.add)
            nc.sync.dma_start(out=lgout[tt * P:(tt + 1) * P, :], in_=lg[:])
            negmx = routr.tile([P, 1], F32, tag="negmx")
            nc.vector.tensor_reduce(out=negmx[:], in_=lg[:], axis=AX.X, op=OP.max,
                                    negate=True)
            ex = routr.tile([P, E], F32, tag="ex")
            nc.scalar.activation(out=ex[:], in_=lg[:], func=ACT.Exp,
                                 bias=negmx[:, :])
            sm = routr.tile([P, 1], F32, tag="sm")
            nc.vector.tensor_reduce(out=sm[:], in_=ex[:], axis=AX.X, op=OP.add)
            rcp = routr.tile([P, 1], F32, tag="rcp")
            nc.vector.reciprocal(out=rcp[:], in_=sm[:])
            pr = routr.tile([P, E], F32, tag="pr")
            nc.vector.tensor_scalar_mul(pr[:], ex[:], rcp[:, :])
            nc.sync.dma_start(out=prout[tt * P:(tt + 1) * P, :], in_=pr[:])
            # one-hot of argmax (exact compare against own max)
            mx = routr.tile([P, 1], F32, tag="mx")
            nc.vector.tensor_reduce(out=mx[:], in_=lg[:], axis=AX.X, op=OP.max)
            oh = routr.tile([P, E], F32, tag="oh")
            nc.vector.tensor_tensor(out=oh[:], in0=lg[:],
                                    in1=mx[:].to_broadcast([P, E]),
                                    op=OP.is_equal)
            nc.sync.dma_start(out=ag_in[tt * P:(tt + 1) * P, :], in_=oh[:])

        # ---- phase 2: all-gather one-hot masks ----
        ag_out = dram.tile([NTOK, E], F32)
        nc.gpsimd.collective_compute(
            "AllGather", OP.bypass,
            replica_groups=[list(range(NCORES))],
            ins=[ag_in[:].opt()],
            outs=[ag_out[:].opt()],
        )

        # ---- phase 3: mask + compaction ----
        ohg = small.tile([P, NI, E], F32, tag="ohg")
        nc.sync.dma_start(out=ohg[:], in_=ag_out[:].rearrange("(i p) e -> p i e", p=P))
        ohsel = small.tile([P, NI, E], F32, tag="ohsel")
        nc.vector.tensor_tensor(out=ohsel[:], in0=ohg[:],
                                in1=sel_sb[:].to_broadcast([P, NI, E]), op=OP.mult)
        mask = small.tile([P, NI], F32, tag="mask")
        nc.vector.tensor_reduce(out=mask[:], in_=ohsel[:], axis=AX.X, op=OP.add)
        cum = small.tile([P, NI], F32, tag="cum")
        nc.vector.tensor_tensor_scan(out=cum[:], data0=mask[:], data1=zeros32[:],
                                     initial=0.0, op0=OP.add, op1=OP.max)
        pspp = psT.tile([P, 1], F32, space="PSUM", tag="pst")
        nc.tensor.matmul(pspp[:], lhsT=ut[:], rhs=cum[:, NI - 1:NI],
                         start=True, stop=True)
        pp = small.tile([P, 1], F32, tag="pp")
        nc.vector.tensor_copy(out=pp[:], in_=pspp[:])
        slot = small.tile([P, NI], F32, tag="slot")
        nc.vector.tensor_tensor(out=slot[:], in0=cum[:], in1=mask[:], op=OP.subtract)
        nc.vector.tensor_tensor(out=slot[:], in0=slot[:],
                                in1=pp[:].to_broadcast([P, NI]), op=OP.add)
        # slots for masked-out tokens -> OOB (skipped by bounds_check)
        nc.vector.tensor_scalar_add(slot[:], slot[:], -OOB_PAD)
        nc.vector.tensor_tensor(out=slot[:], in0=slot[:], in1=mask[:], op=OP.mult)
        nc.vector.tensor_scalar_add(slot[:], slot[:], OOB_PAD)
        slot_i = small.tile([P, NI], I32, tag="slot_i")
        nc.vector.tensor_copy(out=slot_i[:], in_=slot[:])
        tokid = small.tile([P, NI], I32, tag="tokid")
        nc.gpsimd.iota(tokid[:], pattern=[[P, NI]], base=0, channel_multiplier=1)

        permtab = dram.tile([C, 1], I32)
        pm_init = small.tile([P, NST], I32, tag="pm_init")
        nc.vector.memset(pm_init[:], NTOK)   # pad rows point past the end
        nc.sync.dma_start(out=permtab[:].rearrange("(j p) one -> p (j one)", p=P),
                          in_=pm_init[:])
        for i in range(NI):
            nc.gpsimd.indirect_dma_start(
                out=permtab[:, :],
                out_offset=IndirectOffsetOnAxis(ap=slot_i[:, i:i + 1], axis=0),
                in_=tokid[:, i:i + 1],
                in_offset=None,
                bounds_check=C - 1, oob_is_err=False,
            )
        gidx = small.tile([P, NST], I32, tag="gidx")
        nc.sync.dma_start(out=gidx[:],
                          in_=permtab[:].rearrange("(j p) one -> p (j one)", p=P))
        # clamped copy for the gather (pad rows read token NTOK-1 harmlessly)
        gf = small.tile([P, NST], F32, tag="gf")
        nc.vector.tensor_copy(out=gf[:], in_=gidx[:])
        nc.vector.tensor_scalar_min(gf[:], gf[:], float(NTOK - 1))
        gci = small.tile([P, NST], I32, tag="gci")
        nc.vector.tensor_copy(out=gci[:], in_=gf[:])

        # ---- phase 4: gather x rows, transpose to [d, slot] ----
        xgT = xg_pool.tile([P, ND, C], F32R)
        for st in range(NST):
            xg = xs_pool.tile([P, D], F32, tag="xg")
            nc.gpsimd.indirect_dma_start(
                out=xg[:], out_offset=None,
                in_=x[:, :],
                in_offset=IndirectOffsetOnAxis(ap=gci[:, st:st + 1], axis=0),
            )
            for dt_ in range(ND):
                pst = psT.tile([P, P], F32, space="PSUM", tag="pst")
                nc.tensor.transpose(out=pst[:], in_=xg[:, dt_ * P:(dt_ + 1) * P],
                                    identity=ident[:])
                nc.vector.tensor_copy(out=xgT[:, dt_, st * P:(st + 1) * P],
                                      in_=pst[:])

        # ---- phase 5: layer 1 (hT[f, slot] = gelu(W1.T @ xgT + b1)) ----
        hT = hT_pool.tile([P, NF, C], F32R)
        for ft in range(NF):
            psh = psA.tile([P, 2, 512], F32, space="PSUM")
            for dt_ in range(ND):
                w1_sb = wstream.tile([P, P], F32R, tag="w1")
                nc.sync.dma_start(out=w1_sb[:], in_=w1t[dt_, ft, :, :].bitcast(F32R))
                for (cb, c0, cn) in CHUNKS:
                    nc.tensor.matmul(
                        psh[:, cb, 0:cn],
                        lhsT=w1_sb[:],
                        rhs=xgT[:, dt_, c0:c0 + cn],
                        start=(dt_ == 0), stop=(dt_ == ND - 1),
                    )
            for (cb, c0, cn) in CHUNKS:
                nc.scalar.activation(out=hT[:, ft, c0:c0 + cn], in_=psh[:, cb, 0:cn],
                                     func=act_fn, bias=b1_sb[:, ft:ft + 1])

        # ---- phase 6: layer 2 (yT[d, slot] = W2.T @ hT + b2), transpose, scatter
        y_tiles = [y_pool.tile([P, D], F32, tag=f"y{st}", name=f"y{st}")
                   for st in range(NST)]
        for dt_ in range(ND):
            psy = psB.tile([P, 2, 512], F32, space="PSUM")
            for ft in range(NF):
                w2_sb = wstream.tile([P, P], F32R, tag="w2")
                nc.sync.dma_start(out=w2_sb[:], in_=w2t[dt_, ft, :, :].bitcast(F32R))
                for (cb, c0, cn) in CHUNKS:
                    nc.tensor.matmul(
                        psy[:, cb, 0:cn],
                        lhsT=w2_sb[:],
                        rhs=hT[:, ft, c0:c0 + cn],
                        start=(ft == 0), stop=(ft == NF - 1),
                    )
            yT = small.tile([P, C], F32, tag="yT")
            for (cb, c0, cn) in CHUNKS:
                nc.vector.tensor_scalar_add(yT[:, c0:c0 + cn], psy[:, cb, 0:cn],
                                            b2_sb[:, dt_:dt_ + 1])
            for st in range(NST):
                pst = psT.tile([P, P], F32, space="PSUM", tag="pst")
                nc.tensor.transpose(out=pst[:], in_=yT[:, st * P:(st + 1) * P],
                                    identity=ident[:])
                nc.vector.tensor_copy(out=y_tiles[st][:, dt_ * P:(dt_ + 1) * P],
                                      in_=pst[:])
        for st in range(NST):
            nc.gpsimd.indirect_dma_start(
                out=out[:, :],
                out_offset=IndirectOffsetOnAxis(ap=gidx[:, st:st + 1], axis=0),
                in_=y_tiles[st][:],
                in_offset=None,
                bounds_check=NTOK - 1, oob_is_err=False,
            )


_NC_CACHE = None


def _get_nc():
    global _NC_CACHE
    if _NC_CACHE is None:
        _NC_CACHE = build_nc()
    return _NC_CACHE


def make_in_maps(x, router_W, router_b, W1, b1, W2, b2):
    xf = np.ascontiguousarray(x.reshape(NTOK, D).astype(np.float32))
    rwt = np.ascontiguousarray(router_W.astype(np.float32).T)          # [D, E]
    rbv = router_b.astype(np.float32).reshape(1, E)
    in_maps = []
    for c in range(NCORES):
        w1c = W1[c].astype(np.float32)                                  # [D, F]
        w2c = W2[c].astype(np.float32)                                  # [F, D]
        w1tile = np.ascontiguousarray(
            w1c.reshape(ND, P, NF, P).transpose(0, 2, 1, 3))            # [nd,nf,128d,128f]
        w2tile = np.ascontiguousarray(
            w2c.reshape(NF, P, ND, P).transpose(2, 0, 1, 3))            # [nd,nf,128f,128d]
        selv = np.zeros((1, 1, E), np.float32)
        selv[0, 0, c] = 1.0
        in_maps.append({
            "x": xf,
            "xsl": np.ascontiguousarray(xf[c * SLICE:(c + 1) * SLICE]),
            "rwt": rwt,
            "rb": rbv,
            "w1t": w1tile,
            "b1": b1[c].astype(np.float32).reshape(F, 1),
            "w2t": w2tile,
            "b2": b2[c].astype(np.float32).reshape(D, 1),
            "sel": selv,
        })
    return in_maps


def combine_results(results):
    out = np.zeros((NTOK, D), np.float32)
    logits = np.empty((NTOK, E), np.float32)
    probs = np.empty((NTOK, E), np.float32)
    for c, r in enumerate(results):
        out += r["out"]
        logits[c * SLICE:(c + 1) * SLICE] = r["lgout"]
        probs[c * SLICE:(c + 1) * SLICE] = r["prout"]
    return (out.reshape(2, 2048, D),
            probs.reshape(2, 2048, E),
            logits.reshape(2, 2048, E))


def kernel(x, router_W, router_b, W1, b1, W2, b2):
    from concourse.bass_utils import run_bass_kernel_spmd
    nc = _get_nc()
    in_maps = make_in_maps(x, router_W, router_b, W1, b1, W2, b2)
    res = run_bass_kernel_spmd(nc, in_maps, core_ids=list(range(NCORES)))
    return combine_results(res.results)


# revision 26
# speedup vs baseline: 1.1002x; 1.1002x over previous
"""Expert-parallel top-1 MoE FFN kernel for 8 Trainium2 NeuronCores.

Problem: x[2,2048,1024] routed (top-1 of softmax(x @ rW.T + rb)) through one of
E=8 expert FFNs (Linear(1024,4096) -> gelu -> Linear(4096,1024)).
Returns (expert_outputs, router_probs, router_logits).

Strategy (one expert per core, fully independent cores — no collectives;
an NRT start barrier + AllGather measured ~84us, more than recomputing the
router locally):
  - every core computes the full router (all 4096 tokens) with PE-transposed
    x tiles and exact fp32 matmuls; builds its own expert's mask column per
    token tile (t = p + 128*i layout, x's natural 128-row tiles)
  - compacts its expert's token ids with an on-device prefix-sum
    (tensor_tensor_scan + triangular matmul) and per-column indirect-DMA
    scatters into a DRAM perm table (pad slots -> OOB, skipped)
  - indirect-DMA gather of those <=C=640 token rows, FFN with float32r
    matmuls; W1/W2 are host-retiled so they stream from HBM exactly once in
    512KB/1MB contiguous chunks; indirect-DMA scatter of outputs
  - host sums the 8 disjoint per-core outputs; router outputs from core 0
"""

import numpy as np

import concourse.bass as bass
import concourse.tile as tile
from concourse import bacc, mybir
from concourse.bass import IndirectOffsetOnAxis
from concourse.masks import make_identity, make_upper_triangular

F32 = mybir.dt.float32
F32R = mybir.dt.float32r
I32 = mybir.dt.int32
AX = mybir.AxisListType
OP = mybir.AluOpType
ACT = mybir.ActivationFunctionType

P = 128
NTOK = 4096
D = 1024
F = 4096
E = 8
NCORES = 8
SLICE = NTOK // NCORES          # router tokens per core
C = 640                         # expert token capacity per core
CHUNKS = [(0, 0, 384), (1, 384, 256)]  # (psum bank, slot offset, width);
                                       # each >=256 wide for f32r full rate
NI = NTOK // P                  # 32 tokens per partition (t = p*32 + i)
ND = D // P                     # 8
NF = F // P                     # 32
NST = C // P                    # 5 slot tiles
OOB_PAD = 100000.0              # slot value for tokens not on this expert


def build_nc(act_fn=None):
    nc = bacc.Bacc("TRN2", target_bir_lowering=False, debug=False,
                   num_devices=NCORES)

    x = nc.dram_tensor("x", [NTOK, D], F32, kind="ExternalInput").ap()
    rwt = nc.dram_tensor("rwt", [D, E], F32, kind="ExternalInput").ap()
    rb = nc.dram_tensor("rb", [1, E], F32, kind="ExternalInput").ap()
    w1t = nc.dram_tensor("w1t", [NF, P, ND, P], F32, kind="ExternalInput").ap()
    b1 = nc.dram_tensor("b1", [F, 1], F32, kind="ExternalInput").ap()
    w2t = nc.dram_tensor("w2t", [ND, P, NF, P], F32, kind="ExternalInput").ap()
    b2 = nc.dram_tensor("b2", [D, 1], F32, kind="ExternalInput").ap()
    sel = nc.dram_tensor("sel", [1, 1, E], F32, kind="ExternalInput").ap()

    out = nc.dram_tensor("out", [NTOK, D], F32, kind="ExternalOutput").ap()
    lgout = nc.dram_tensor("lgout", [NTOK, E], F32, kind="ExternalOutput").ap()
    prout = nc.dram_tensor("prout", [NTOK, E], F32, kind="ExternalOutput").ap()

    with tile.TileContext(nc) as tc:
        build_kernel(tc, x, rwt, rb, w1t, b1, w2t, b2, sel,
                     out, lgout, prout, act_fn=act_fn or ACT.Gelu)
    nc.compile()
    return nc


def build_kernel(tc, x, rwt, rb, w1t, b1, w2t, b2, sel,
                 out, lgout, prout, act_fn=ACT.Gelu):
    nc = tc.nc
    with (
        tc.tile_pool(name="const", bufs=1) as const,
        tc.tile_pool(name="small", bufs=2) as small,
        tc.tile_pool(name="routr", bufs=4) as routr,
        tc.tile_pool(name="xg", bufs=1) as xg_pool,
        tc.tile_pool(name="hT", bufs=1) as hT_pool,
        tc.tile_pool(name="w1p", bufs=3) as w1p,
        tc.tile_pool(name="w2p", bufs=2) as w2p,
        tc.tile_pool(name="yy", bufs=1) as y_pool,
        tc.tile_pool(name="psA", bufs=2, space="PSUM") as psA,      # 2x2 banks
        tc.tile_pool(name="psB", bufs=1, space="PSUM") as psB,      # 1x2 banks
        tc.tile_pool(name="psT", bufs=2, space="PSUM") as psT,      # 2x1 banks
        tc.tile_pool(name="dram", bufs=1, space="DRAM") as dram,
    ):
        # ---- constants ----
        ident = const.tile([P, P], F32)
        make_identity(nc, ident[:])
        ut = const.tile([P, P], F32)                  # strictly-upper ones
        make_upper_triangular(nc, ut[:], val=1.0, diag=False)
        rwt_sb = const.tile([P, ND, E], F32)
        nc.sync.dma_start(out=rwt_sb[:], in_=rwt.rearrange("(dt p) e -> p dt e", p=P))
        rb_sb = const.tile([P, E], F32)
        nc.sync.dma_start(out=rb_sb[:], in_=rb.to_broadcast([P, E]))
        sel_sb = const.tile([P, 1, E], F32)
        nc.sync.dma_start(out=sel_sb[:], in_=sel.to_broadcast([P, 1, E]))
        b1_sb = const.tile([P, NF], F32)
        nc.sync.dma_start(out=b1_sb[:], in_=b1.rearrange("(ft p) one -> p (ft one)", p=P))
        b2_sb = const.tile([P, ND], F32)
        nc.sync.dma_start(out=b2_sb[:], in_=b2.rearrange("(dt p) one -> p (dt one)", p=P))
        zeros32 = const.tile([P, NI], F32)
        nc.vector.memset(zeros32[:], 0.0)

        permtab = dram.tile([C, 1], I32)

        with (
            tc.tile_pool(name="xs", bufs=3) as xs_pool,
            tc.tile_pool(name="xgp", bufs=2) as xgrow_pool,
            tc.tile_pool(name="xT", bufs=3) as xT_pool,
        ):
            # ---- phase 1: full router on every core (t = p + 128*i) ----
            mask = small.tile([P, NI], F32, tag="mask")
            NCH = 2          # token tiles per x DMA chunk (1MB)
            for ch in range(NI // NCH):
                xs = xs_pool.tile([P, NCH, D], F32, tag="xs")
                nc.sync.dma_start(
                    out=xs[:],
                    in_=x[ch * NCH * P:(ch + 1) * NCH * P, :]
                        .rearrange("(tt p) d -> p tt d", p=P))
                for sub in range(NCH):
                    i = ch * NCH + sub
                    xT = xT_pool.tile([P, ND, P], F32, tag="xT")
                    for dt_ in range(ND):
                        pst = psT.tile([P, P], F32, space="PSUM", tag="pst")
                        nc.tensor.transpose(out=pst[:],
                                            in_=xs[:, sub, dt_ * P:(dt_ + 1) * P],
                                            identity=ident[:])
                        if dt_ % 2 == 0:
                            nc.vector.tensor_copy(out=xT[:, dt_, :], in_=pst[:])
                        else:
                            nc.scalar.copy(out=xT[:, dt_, :], in_=pst[:])
                    psl = psT.tile([P, E], F32, space="PSUM", tag="pst")
                    for dt_ in range(ND):
                        nc.tensor.matmul(psl[:], lhsT=xT[:, dt_, :],
                                         rhs=rwt_sb[:, dt_, :],
                                         start=(dt_ == 0), stop=(dt_ == ND - 1))
                    lg = routr.tile([P, E], F32, tag="lg")
                    nc.vector.tensor_tensor(out=lg[:], in0=psl[:], in1=rb_sb[:],
                                            op=OP.add)
                    nc.sync.dma_start(out=lgout[i * P:(i + 1) * P, :], in_=lg[:])
                    # own-expert mask column (critical path)
                    mx = routr.tile([P, 1], F32, tag="mx")
                    nc.vector.tensor_reduce(out=mx[:], in_=lg[:], axis=AX.X,
                                            op=OP.max)
                    oh = routr.tile([P, E], F32, tag="oh")
                    nc.vector.tensor_tensor(out=oh[:], in0=lg[:],
                                            in1=mx[:].to_broadcast([P, E]),
                                            op=OP.is_equal)
                    ohs = routr.tile([P, E], F32, tag="ohs")
                    nc.vector.tensor_tensor(out=ohs[:], in0=oh[:],
                                            in1=sel_sb[:, 0, :], op=OP.mult)
                    nc.vector.tensor_reduce(out=mask[:, i:i + 1], in_=ohs[:],
                                            axis=AX.X, op=OP.add)
                    # softmax (off critical path)
                    negmx = routr.tile([P, 1], F32, tag="negmx")
                    nc.vector.tensor_reduce(out=negmx[:], in_=lg[:], axis=AX.X,
                                            op=OP.max, negate=True)
                    ex = routr.tile([P, E], F32, tag="ex")
                    nc.scalar.activation(out=ex[:], in_=lg[:], func=ACT.Exp,
                                         bias=negmx[:, :])
                    sm = routr.tile([P, 1], F32, tag="sm")
                    nc.vector.tensor_reduce(out=sm[:], in_=ex[:], axis=AX.X,
                                            op=OP.add)
                    rcp = routr.tile([P, 1], F32, tag="rcp")
                    nc.vector.reciprocal(out=rcp[:], in_=sm[:])
                    pr = routr.tile([P, E], F32, tag="pr")
                    nc.vector.tensor_scalar_mul(pr[:], ex[:], rcp[:, :])
                    nc.sync.dma_start(out=prout[i * P:(i + 1) * P, :], in_=pr[:])

            # ---- phase 3: compaction ----
            cum = small.tile([P, NI], F32, tag="cum")
            nc.vector.tensor_tensor_scan(out=cum[:], data0=mask[:], data1=zeros32[:],
                                         initial=0.0, op0=OP.add, op1=OP.max)
            pspp = psT.tile([P, 1], F32, space="PSUM", tag="pst")
            nc.tensor.matmul(pspp[:], lhsT=ut[:], rhs=cum[:, NI - 1:NI],
                             start=True, stop=True)
            pp = small.tile([P, 1], F32, tag="pp")
            nc.vector.tensor_copy(out=pp[:], in_=pspp[:])
            slot = small.tile([P, NI], F32, tag="slot")
            nc.vector.tensor_tensor(out=slot[:], in0=cum[:], in1=mask[:],
                                    op=OP.subtract)
            nc.vector.tensor_tensor(out=slot[:], in0=slot[:],
                                    in1=pp[:].to_broadcast([P, NI]), op=OP.add)
            # slots for masked-out tokens -> OOB (skipped by bounds_check)
            nc.vector.tensor_scalar_add(slot[:], slot[:], -OOB_PAD)
            nc.vector.tensor_tensor(out=slot[:], in0=slot[:], in1=mask[:],
                                    op=OP.mult)
            nc.vector.tensor_scalar_add(slot[:], slot[:], OOB_PAD)
            slot_i = small.tile([P, NI], I32, tag="slot_i")
            nc.vector.tensor_copy(out=slot_i[:], in_=slot[:])
            tokid = small.tile([P, NI], I32, tag="tokid")
            nc.gpsimd.iota(tokid[:], pattern=[[P, NI]], base=0,
                           channel_multiplier=1)

            pm_init = small.tile([P, NST], I32, tag="pm_init")
            nc.vector.memset(pm_init[:], NTOK)   # pad rows point past the end
            nc.sync.dma_start(
                out=permtab[:].rearrange("(j p) one -> p (j one)", p=P),
                in_=pm_init[:])
            # NB: multi-column offset APs are wrong on HW (sim-only semantics);
            # scatter one [128,1] column per call.
            for i in range(NI):
                nc.gpsimd.indirect_dma_start(
                    out=permtab[:, :],
                    out_offset=IndirectOffsetOnAxis(ap=slot_i[:, i:i + 1], axis=0),
                    in_=tokid[:, i:i + 1],
                    in_offset=None,
                    bounds_check=C - 1, oob_is_err=False,
                )
            gidx = small.tile([P, NST], I32, tag="gidx")
            nc.sync.dma_start(
                out=gidx[:],
                in_=permtab[:].rearrange("(j p) one -> p (j one)", p=P))
            # clamped copy for the gather (pad rows read token NTOK-1 harmlessly)
            gf = small.tile([P, NST], F32, tag="gf")
            nc.vector.tensor_copy(out=gf[:], in_=gidx[:])
            nc.vector.tensor_scalar_min(gf[:], gf[:], float(NTOK - 1))
            gci = small.tile([P, NST], I32, tag="gci")
            nc.vector.tensor_copy(out=gci[:], in_=gf[:])

            # ---- phase 4: gather x rows, transpose to [d, slot] ----
            xgT = xg_pool.tile([P, ND, C], F32R)
            for st in range(NST):
                xg = xgrow_pool.tile([P, D], F32, tag="xg")
                nc.gpsimd.indirect_dma_start(
                    out=xg[:], out_offset=None,
                    in_=x[:, :],
                    in_offset=IndirectOffsetOnAxis(ap=gci[:, st:st + 1], axis=0),
                )
                for dt_ in range(ND):
                    pst = psT.tile([P, P], F32, space="PSUM", tag="pst")
                    nc.tensor.transpose(out=pst[:],
                                        in_=xg[:, dt_ * P:(dt_ + 1) * P],
                                        identity=ident[:])
                    nc.vector.tensor_copy(out=xgT[:, dt_, st * P:(st + 1) * P],
                                          in_=pst[:])

        # ---- phase 5: layer 1 (hT[f, slot] = gelu(W1.T @ xgT + b1)) ----
        hT = hT_pool.tile([P, NF, C], F32R)
        for ft in range(NF):
            w1row = w1p.tile([P, ND, P], F32R, tag="w1")
            nc.sync.dma_start(out=w1row[:], in_=w1t[ft].bitcast(F32R))
            psh = psA.tile([P, 2, 512], F32, space="PSUM")
            for dt_ in range(ND):
                for (cb, c0, cn) in CHUNKS:
                    nc.tensor.matmul(
                        psh[:, cb, 0:cn],
                        lhsT=w1row[:, dt_, :],
                        rhs=xgT[:, dt_, c0:c0 + cn],
                        start=(dt_ == 0), stop=(dt_ == ND - 1),
                    )
            for (cb, c0, cn) in CHUNKS:
                nc.scalar.activation(out=hT[:, ft, c0:c0 + cn], in_=psh[:, cb, 0:cn],
                                     func=act_fn, bias=b1_sb[:, ft:ft + 1])

        # ---- phase 6: layer 2 (yT[d, slot] = W2.T @ hT + b2), transpose, scatter
        y_tiles = [y_pool.tile([P, D], F32, tag=f"y{st}", name=f"y{st}")
                   for st in range(NST)]
        NFH = NF // 2
        for dt_ in range(ND):
            psy = psB.tile([P, 2, 512], F32, space="PSUM")
            for half in range(2):
                w2row = w2p.tile([P, NFH, P], F32R, tag="w2")
                nc.sync.dma_start(
                    out=w2row[:],
                    in_=w2t[dt_, :, half * NFH:(half + 1) * NFH, :].bitcast(F32R))
                for fh in range(NFH):
                    ft = half * NFH + fh
                    for (cb, c0, cn) in CHUNKS:
                        nc.tensor.matmul(
                            psy[:, cb, 0:cn],
                            lhsT=w2row[:, fh, :],
                            rhs=hT[:, ft, c0:c0 + cn],
                            start=(ft == 0), stop=(ft == NF - 1),
                        )
            yT = small.tile([P, C], F32, tag="yT")
            for (cb, c0, cn) in CHUNKS:
                nc.vector.tensor_scalar_add(yT[:, c0:c0 + cn], psy[:, cb, 0:cn],
                                            b2_sb[:, dt_:dt_ + 1])
            for st in range(NST):
                pst = psT.tile([P, P], F32, space="PSUM", tag="pst")
                nc.tensor.transpose(out=pst[:], in_=yT[:, st * P:(st + 1) * P],
                                    identity=ident[:])
                nc.vector.tensor_copy(out=y_tiles[st][:, dt_ * P:(dt_ + 1) * P],
                                      in_=pst[:])
        for st in range(NST):
            nc.gpsimd.indirect_dma_start(
                out=out[:, :],
                out_offset=IndirectOffsetOnAxis(ap=gidx[:, st:st + 1], axis=0),
                in_=y_tiles[st][:],
                in_offset=None,
                bounds_check=NTOK - 1, oob_is_err=False,
            )


_NC_CACHE = None


def _get_nc():
    global _NC_CACHE
    if _NC_CACHE is None:
        _NC_CACHE = build_nc()
    return _NC_CACHE


def make_in_maps(x, router_W, router_b, W1, b1, W2, b2):
    xf = np.ascontiguousarray(np.asarray(x).reshape(NTOK, D).astype(np.float32))
    rwt = np.ascontiguousarray(np.asarray(router_W).astype(np.float32).T)  # [D, E]
    rbv = np.asarray(router_b).astype(np.float32).reshape(1, E)
    in_maps = []
    for c in range(NCORES):
        w1c = np.asarray(W1[c]).astype(np.float32)                  # [D, F]
        w2c = np.asarray(W2[c]).astype(np.float32)                  # [F, D]
        # w1t[ft, p, dt, j] = W1[dt*128+p, ft*128+j]
        w1tile = np.ascontiguousarray(
            w1c.reshape(ND, P, NF, P).transpose(2, 1, 0, 3))
        # w2t[dt, p, ft, j] = W2[ft*128+p, dt*128+j]
        w2tile = np.ascontiguousarray(
            w2c.reshape(NF, P, ND, P).transpose(2, 1, 0, 3))
        selv = np.zeros((1, 1, E), np.float32)
        selv[0, 0, c] = 1.0
        in_maps.append({
            "x": xf,
            "rwt": rwt,
            "rb": rbv,
            "w1t": w1tile,
            "b1": np.asarray(b1[c]).astype(np.float32).reshape(F, 1),
            "w2t": w2tile,
            "b2": np.asarray(b2[c]).astype(np.float32).reshape(D, 1),
            "sel": selv,
        })
    return in_maps


def combine_results(results):
    out = np.zeros((NTOK, D), np.float32)
    for r in results:
        out += r["out"]
    logits = results[0]["lgout"]
    probs = results[0]["prout"]
    return (out.reshape(2, 2048, D),
            probs.reshape(2, 2048, E),
            logits.reshape(2, 2048, E))


def kernel(x, router_W, router_b, W1, b1, W2, b2):
    from concourse.bass_utils import run_bass_kernel_spmd
    nc = _get_nc()
    in_maps = make_in_maps(x, router_W, router_b, W1, b1, W2, b2)
    res = run_bass_kernel_spmd(nc, in_maps, core_ids=list(range(NCORES)))
    return combine_results(res.results)


# revision 27
# speedup vs baseline: 1.3281x; 1.2072x over previous
"""Expert-parallel top-1 MoE FFN kernel for 8 Trainium2 NeuronCores.

Problem: x[2,2048,1024] routed (top-1 of softmax(x @ rW.T + rb)) through one of
E=8 expert FFNs (Linear(1024,4096) -> gelu -> Linear(4096,1024)).
Returns (expert_outputs, router_probs, router_logits).

Strategy (one expert per core, fully independent cores — no collectives;
an NRT start barrier + AllGather measured ~84us, more than recomputing the
router locally):
  - every core computes the full router (all 4096 tokens) from a
    host-transposed copy of x (xt = x.T, layout prep in the host wrapper):
    64 dense fp32 matmuls with the tiny router weights stationary produce
    logitsT[8, tok]; small PE transposes flip each 128-token tile back to
    [tok, 8] for softmax/argmax (t = p + 128*i layout)
  - compacts its expert's token ids with an on-device prefix-sum
    (tensor_tensor_scan + triangular matmul) and per-column indirect-DMA
    scatters into a DRAM perm table (pad slots -> OOB, skipped)
  - indirect-DMA gather of those <=C=640 token rows, FFN with float32r
    matmuls; W1/W2 are host-retiled so they stream from HBM exactly once in
    512KB/1MB contiguous chunks; indirect-DMA scatter of outputs
  - host sums the 8 disjoint per-core outputs; router outputs from core 0
"""

import numpy as np

import concourse.bass as bass
import concourse.tile as tile
from concourse import bacc, mybir
from concourse.bass import IndirectOffsetOnAxis
from concourse.masks import make_identity, make_upper_triangular

F32 = mybir.dt.float32
F32R = mybir.dt.float32r
I32 = mybir.dt.int32
AX = mybir.AxisListType
OP = mybir.AluOpType
ACT = mybir.ActivationFunctionType

P = 128
NTOK = 4096
D = 1024
F = 4096
E = 8
NCORES = 8
SLICE = NTOK // NCORES          # router tokens per core
C = 640                         # expert token capacity per core
CHUNKS = [(0, 0, 384), (1, 384, 256)]  # (psum bank, slot offset, width);
                                       # each >=256 wide for f32r full rate
NI = NTOK // P                  # 32 tokens per partition (t = p*32 + i)
ND = D // P                     # 8
NF = F // P                     # 32
NST = C // P                    # 5 slot tiles
OOB_PAD = 100000.0              # slot value for tokens not on this expert


def build_nc(act_fn=None):
    nc = bacc.Bacc("TRN2", target_bir_lowering=False, debug=False,
                   num_devices=NCORES)

    x = nc.dram_tensor("x", [NTOK, D], F32, kind="ExternalInput").ap()
    xt = nc.dram_tensor("xt", [D, NTOK], F32, kind="ExternalInput").ap()
    rwt = nc.dram_tensor("rwt", [D, E], F32, kind="ExternalInput").ap()
    rb = nc.dram_tensor("rb", [1, E], F32, kind="ExternalInput").ap()
    rbt = nc.dram_tensor("rbt", [E, 1], F32, kind="ExternalInput").ap()
    w1t = nc.dram_tensor("w1t", [NF, P, ND, P], F32, kind="ExternalInput").ap()
    b1 = nc.dram_tensor("b1", [F, 1], F32, kind="ExternalInput").ap()
    w2t = nc.dram_tensor("w2t", [ND, P, NF, P], F32, kind="ExternalInput").ap()
    b2 = nc.dram_tensor("b2", [D, 1], F32, kind="ExternalInput").ap()
    sel = nc.dram_tensor("sel", [1, 1, E], F32, kind="ExternalInput").ap()

    out = nc.dram_tensor("out", [NTOK, D], F32, kind="ExternalOutput").ap()
    lgout = nc.dram_tensor("lgout", [NTOK, E], F32, kind="ExternalOutput").ap()
    prout = nc.dram_tensor("prout", [NTOK, E], F32, kind="ExternalOutput").ap()

    with tile.TileContext(nc) as tc:
        build_kernel(tc, x, xt, rwt, rb, rbt, w1t, b1, w2t, b2, sel,
                     out, lgout, prout, act_fn=act_fn or ACT.Gelu)
    nc.compile()
    return nc


def build_kernel(tc, x, xt, rwt, rb, rbt, w1t, b1, w2t, b2, sel,
                 out, lgout, prout, act_fn=ACT.Gelu):
    nc = tc.nc
    with (
        tc.tile_pool(name="const", bufs=1) as const,
        tc.tile_pool(name="small", bufs=2) as small,
        tc.tile_pool(name="routr", bufs=4) as routr,
        tc.tile_pool(name="xg", bufs=1) as xg_pool,
        tc.tile_pool(name="hT", bufs=1) as hT_pool,
        tc.tile_pool(name="w1p", bufs=3) as w1p,
        tc.tile_pool(name="w2p", bufs=2) as w2p,
        tc.tile_pool(name="yy", bufs=1) as y_pool,
        tc.tile_pool(name="psA", bufs=2, space="PSUM") as psA,      # 2x2 banks
        tc.tile_pool(name="psB", bufs=1, space="PSUM") as psB,      # 1x2 banks
        tc.tile_pool(name="psT", bufs=2, space="PSUM") as psT,      # 2x1 banks
        tc.tile_pool(name="dram", bufs=1, space="DRAM") as dram,
    ):
        # ---- constants ----
        ident = const.tile([P, P], F32)
        make_identity(nc, ident[:])
        ut = const.tile([P, P], F32)                  # strictly-upper ones
        make_upper_triangular(nc, ut[:], val=1.0, diag=False)
        rwt_sb = const.tile([P, ND, E], F32)
        nc.sync.dma_start(out=rwt_sb[:], in_=rwt.rearrange("(dt p) e -> p dt e", p=P))
        rb_sb = const.tile([P, E], F32)
        nc.sync.dma_start(out=rb_sb[:], in_=rb.to_broadcast([P, E]))
        rbt_sb = const.tile([E, 1], F32)
        nc.sync.dma_start(out=rbt_sb[:], in_=rbt[:, :])
        sel_sb = const.tile([P, 1, E], F32)
        nc.sync.dma_start(out=sel_sb[:], in_=sel.to_broadcast([P, 1, E]))
        b1_sb = const.tile([P, NF], F32)
        nc.sync.dma_start(out=b1_sb[:], in_=b1.rearrange("(ft p) one -> p (ft one)", p=P))
        b2_sb = const.tile([P, ND], F32)
        nc.sync.dma_start(out=b2_sb[:], in_=b2.rearrange("(dt p) one -> p (dt one)", p=P))
        zeros32 = const.tile([P, NI], F32)
        nc.vector.memset(zeros32[:], 0.0)

        permtab = dram.tile([C, 1], I32)

        with (
            tc.tile_pool(name="xtp", bufs=2) as xt_pool,
            tc.tile_pool(name="xgp", bufs=2) as xgrow_pool,
        ):
            # ---- phase 1: full router on every core (t = p + 128*i) ----
            # logitsT[8, tok] = rwt.T @ xt, router weights stationary.
            mask = small.tile([P, NI], F32, tag="mask")
            TCH = 512        # tokens per router matmul chunk
            for tch in range(NTOK // TCH):
                xt_sb = xt_pool.tile([P, ND, TCH], F32, tag="xt")
                nc.sync.dma_start(
                    out=xt_sb[:],
                    in_=xt[:, tch * TCH:(tch + 1) * TCH]
                        .rearrange("(dt p) t -> p dt t", p=P))
                pslt = psT.tile([E, TCH], F32, space="PSUM", tag="pst")
                for dt_ in range(ND):
                    nc.tensor.matmul(pslt[:], lhsT=rwt_sb[:, dt_, :],
                                     rhs=xt_sb[:, dt_, :],
                                     start=(dt_ == 0), stop=(dt_ == ND - 1))
                lgT = routr.tile([E, TCH], F32, tag="lgT")
                nc.vector.tensor_scalar_add(lgT[:], pslt[:], rbt_sb[:, :])
                for j in range(TCH // P):
                    i = tch * (TCH // P) + j
                    pslg = psT.tile([P, E], F32, space="PSUM", tag="pst")
                    nc.tensor.transpose(out=pslg[:],
                                        in_=lgT[:, j * P:(j + 1) * P],
                                        identity=ident[:E, :E])
                    lg = routr.tile([P, E], F32, tag="lg")
                    nc.vector.tensor_copy(out=lg[:], in_=pslg[:])
                    nc.sync.dma_start(out=lgout[i * P:(i + 1) * P, :], in_=lg[:])
                    # own-expert mask column (critical path)
                    mx = routr.tile([P, 1], F32, tag="mx")
                    nc.vector.tensor_reduce(out=mx[:], in_=lg[:], axis=AX.X,
                                            op=OP.max)
                    oh = routr.tile([P, E], F32, tag="oh")
                    nc.vector.tensor_tensor(out=oh[:], in0=lg[:],
                                            in1=mx[:].to_broadcast([P, E]),
                                            op=OP.is_equal)
                    ohs = routr.tile([P, E], F32, tag="ohs")
                    nc.vector.tensor_tensor(out=ohs[:], in0=oh[:],
                                            in1=sel_sb[:, 0, :], op=OP.mult)
                    nc.vector.tensor_reduce(out=mask[:, i:i + 1], in_=ohs[:],
                                            axis=AX.X, op=OP.add)
                    # softmax (off critical path)
                    negmx = routr.tile([P, 1], F32, tag="negmx")
                    nc.vector.tensor_reduce(out=negmx[:], in_=lg[:], axis=AX.X,
                                            op=OP.max, negate=True)
                    ex = routr.tile([P, E], F32, tag="ex")
                    nc.scalar.activation(out=ex[:], in_=lg[:], func=ACT.Exp,
                                         bias=negmx[:, :])
                    sm = routr.tile([P, 1], F32, tag="sm")
                    nc.vector.tensor_reduce(out=sm[:], in_=ex[:], axis=AX.X,
                                            op=OP.add)
                    rcp = routr.tile([P, 1], F32, tag="rcp")
                    nc.vector.reciprocal(out=rcp[:], in_=sm[:])
                    pr = routr.tile([P, E], F32, tag="pr")
                    nc.vector.tensor_scalar_mul(pr[:], ex[:], rcp[:, :])
                    nc.sync.dma_start(out=prout[i * P:(i + 1) * P, :], in_=pr[:])

            # ---- phase 3: compaction ----
            cum = small.tile([P, NI], F32, tag="cum")
            nc.vector.tensor_tensor_scan(out=cum[:], data0=mask[:], data1=zeros32[:],
                                         initial=0.0, op0=OP.add, op1=OP.max)
            pspp = psT.tile([P, 1], F32, space="PSUM", tag="pst")
            nc.tensor.matmul(pspp[:], lhsT=ut[:], rhs=cum[:, NI - 1:NI],
                             start=True, stop=True)
            pp = small.tile([P, 1], F32, tag="pp")
            nc.vector.tensor_copy(out=pp[:], in_=pspp[:])
            slot = small.tile([P, NI], F32, tag="slot")
            nc.vector.tensor_tensor(out=slot[:], in0=cum[:], in1=mask[:],
                                    op=OP.subtract)
            nc.vector.tensor_tensor(out=slot[:], in0=slot[:],
                                    in1=pp[:].to_broadcast([P, NI]), op=OP.add)
            # slots for masked-out tokens -> OOB (skipped by bounds_check)
            nc.vector.tensor_scalar_add(slot[:], slot[:], -OOB_PAD)
            nc.vector.tensor_tensor(out=slot[:], in0=slot[:], in1=mask[:],
                                    op=OP.mult)
            nc.vector.tensor_scalar_add(slot[:], slot[:], OOB_PAD)
            slot_i = small.tile([P, NI], I32, tag="slot_i")
            nc.vector.tensor_copy(out=slot_i[:], in_=slot[:])
            tokid = small.tile([P, NI], I32, tag="tokid")
            nc.gpsimd.iota(tokid[:], pattern=[[P, NI]], base=0,
                           channel_multiplier=1)

            pm_init = small.tile([P, NST], I32, tag="pm_init")
            nc.vector.memset(pm_init[:], NTOK)   # pad rows point past the end
            nc.sync.dma_start(
                out=permtab[:].rearrange("(j p) one -> p (j one)", p=P),
                in_=pm_init[:])
            # NB: multi-column offset APs are wrong on HW (sim-only semantics);
            # scatter one [128,1] column per call.
            for i in range(NI):
                nc.gpsimd.indirect_dma_start(
                    out=permtab[:, :],
                    out_offset=IndirectOffsetOnAxis(ap=slot_i[:, i:i + 1], axis=0),
                    in_=tokid[:, i:i + 1],
                    in_offset=None,
                    bounds_check=C - 1, oob_is_err=False,
                )
            gidx = small.tile([P, NST], I32, tag="gidx")
            nc.sync.dma_start(
                out=gidx[:],
                in_=permtab[:].rearrange("(j p) one -> p (j one)", p=P))
            # clamped copy for the gather (pad rows read token NTOK-1 harmlessly)
            gf = small.tile([P, NST], F32, tag="gf")
            nc.vector.tensor_copy(out=gf[:], in_=gidx[:])
            nc.vector.tensor_scalar_min(gf[:], gf[:], float(NTOK - 1))
            gci = small.tile([P, NST], I32, tag="gci")
            nc.vector.tensor_copy(out=gci[:], in_=gf[:])

            # ---- phase 4: gather x rows, transpose to [d, slot] ----
            xgT = xg_pool.tile([P, ND, C], F32R)
            for st in range(NST):
                xg = xgrow_pool.tile([P, D], F32, tag="xg")
                nc.gpsimd.indirect_dma_start(
                    out=xg[:], out_offset=None,
                    in_=x[:, :],
                    in_offset=IndirectOffsetOnAxis(ap=gci[:, st:st + 1], axis=0),
                )
                for dt_ in range(ND):
                    pst = psT.tile([P, P], F32, space="PSUM", tag="pst")
                    nc.tensor.transpose(out=pst[:],
                                        in_=xg[:, dt_ * P:(dt_ + 1) * P],
                                        identity=ident[:])
                    nc.vector.tensor_copy(out=xgT[:, dt_, st * P:(st + 1) * P],
                                          in_=pst[:])

        # ---- phase 5: layer 1 (hT[f, slot] = gelu(W1.T @ xgT + b1)) ----
        hT = hT_pool.tile([P, NF, C], F32R)
        for ft in range(NF):
            w1row = w1p.tile([P, ND, P], F32R, tag="w1")
            nc.sync.dma_start(out=w1row[:], in_=w1t[ft].bitcast(F32R))
            psh = psA.tile([P, 2, 512], F32, space="PSUM")
            for dt_ in range(ND):
                for (cb, c0, cn) in CHUNKS:
                    nc.tensor.matmul(
                        psh[:, cb, 0:cn],
                        lhsT=w1row[:, dt_, :],
                        rhs=xgT[:, dt_, c0:c0 + cn],
                        start=(dt_ == 0), stop=(dt_ == ND - 1),
                    )
            for (cb, c0, cn) in CHUNKS:
                nc.scalar.activation(out=hT[:, ft, c0:c0 + cn], in_=psh[:, cb, 0:cn],
                                     func=act_fn, bias=b1_sb[:, ft:ft + 1])

        # ---- phase 6: layer 2 (yT[d, slot] = W2.T @ hT + b2), transpose, scatter
        y_tiles = [y_pool.tile([P, D], F32, tag=f"y{st}", name=f"y{st}")
                   for st in range(NST)]
        NFH = NF // 2
        for dt_ in range(ND):
            psy = psB.tile([P, 2, 512], F32, space="PSUM")
            for half in range(2):
                w2row = w2p.tile([P, NFH, P], F32R, tag="w2")
                nc.sync.dma_start(
                    out=w2row[:],
                    in_=w2t[dt_, :, half * NFH:(half + 1) * NFH, :].bitcast(F32R))
                for fh in range(NFH):
                    ft = half * NFH + fh
                    for (cb, c0, cn) in CHUNKS:
                        nc.tensor.matmul(
                            psy[:, cb, 0:cn],
                            lhsT=w2row[:, fh, :],
                            rhs=hT[:, ft, c0:c0 + cn],
                            start=(ft == 0), stop=(ft == NF - 1),
                        )
            yT = small.tile([P, C], F32, tag="yT")
            for (cb, c0, cn) in CHUNKS:
                nc.vector.tensor_scalar_add(yT[:, c0:c0 + cn], psy[:, cb, 0:cn],
                                            b2_sb[:, dt_:dt_ + 1])
            for st in range(NST):
                pst = psT.tile([P, P], F32, space="PSUM", tag="pst")
                nc.tensor.transpose(out=pst[:], in_=yT[:, st * P:(st + 1) * P],
                                    identity=ident[:])
                nc.vector.tensor_copy(out=y_tiles[st][:, dt_ * P:(dt_ + 1) * P],
                                      in_=pst[:])
        for st in range(NST):
            nc.gpsimd.indirect_dma_start(
                out=out[:, :],
                out_offset=IndirectOffsetOnAxis(ap=gidx[:, st:st + 1], axis=0),
                in_=y_tiles[st][:],
                in_offset=None,
                bounds_check=NTOK - 1, oob_is_err=False,
            )


_NC_CACHE = None


def _get_nc():
    global _NC_CACHE
    if _NC_CACHE is None:
        _NC_CACHE = build_nc()
    return _NC_CACHE


def make_in_maps(x, router_W, router_b, W1, b1, W2, b2):
    xf = np.ascontiguousarray(np.asarray(x).reshape(NTOK, D).astype(np.float32))
    xtf = np.ascontiguousarray(xf.T)
    rwt = np.ascontiguousarray(np.asarray(router_W).astype(np.float32).T)  # [D, E]
    rbv = np.asarray(router_b).astype(np.float32).reshape(1, E)
    in_maps = []
    for c in range(NCORES):
        w1c = np.asarray(W1[c]).astype(np.float32)                  # [D, F]
        w2c = np.asarray(W2[c]).astype(np.float32)                  # [F, D]
        # w1t[ft, p, dt, j] = W1[dt*128+p, ft*128+j]
        w1tile = np.ascontiguousarray(
            w1c.reshape(ND, P, NF, P).transpose(2, 1, 0, 3))
        # w2t[dt, p, ft, j] = W2[ft*128+p, dt*128+j]
        w2tile = np.ascontiguousarray(
            w2c.reshape(NF, P, ND, P).transpose(2, 1, 0, 3))
        selv = np.zeros((1, 1, E), np.float32)
        selv[0, 0, c] = 1.0
        in_maps.append({
            "x": xf,
            "xt": xtf,
            "rwt": rwt,
            "rb": rbv,
            "rbt": rbv.reshape(E, 1).copy(),
            "w1t": w1tile,
            "b1": np.asarray(b1[c]).astype(np.float32).reshape(F, 1),
            "w2t": w2tile,
            "b2": np.asarray(b2[c]).astype(np.float32).reshape(D, 1),
            "sel": selv,
        })
    return in_maps


def combine_results(results):
    out = np.zeros((NTOK, D), np.float32)
    for r in results:
        out += r["out"]
    logits = results[0]["lgout"]
    probs = results[0]["prout"]
    return (out.reshape(2, 2048, D),
            probs.reshape(2, 2048, E),
            logits.reshape(2, 2048, E))


def kernel(x, router_W, router_b, W1, b1, W2, b2):
    from concourse.bass_utils import run_bass_kernel_spmd
    nc = _get_nc()
    in_maps = make_in_maps(x, router_W, router_b, W1, b1, W2, b2)
    res = run_bass_kernel_spmd(nc, in_maps, core_ids=list(range(NCORES)))
    return combine_results(res.results)


# revision 31
# speedup vs baseline: 1.4030x; 1.0564x over previous
"""Expert-parallel top-1 MoE FFN kernel for 8 Trainium2 NeuronCores.

Problem: x[2,2048,1024] routed (top-1 of softmax(x @ rW.T + rb)) through one of
E=8 expert FFNs (Linear(1024,4096) -> gelu -> Linear(4096,1024)).
Returns (expert_outputs, router_probs, router_logits).

Strategy (one expert per core, fully independent cores — no collectives;
an NRT start barrier + AllGather measured ~84us, more than recomputing the
router locally):
  - every core computes the full router (all 4096 tokens) from a
    host-transposed copy of x (xt = x.T, layout prep in the host wrapper):
    64 dense fp32 matmuls with the tiny router weights stationary produce
    logitsT[8, tok]; small PE transposes flip each 128-token tile back to
    [tok, 8] for softmax/argmax (t = p + 128*i layout)
  - compacts its expert's token ids with an on-device prefix-sum
    (tensor_tensor_scan + triangular matmul) and per-column indirect-DMA
    scatters into a DRAM perm table (pad slots -> OOB, skipped)
  - indirect-DMA gather of those <=C=640 token rows, FFN with float32r
    matmuls; W1/W2 are host-retiled so they stream from HBM exactly once in
    512KB/1MB contiguous chunks; indirect-DMA scatter of outputs
  - host sums the 8 disjoint per-core outputs; router outputs from core 0
"""

import numpy as np

import concourse.bass as bass
import concourse.tile as tile
from concourse import bacc, mybir
from concourse.bass import IndirectOffsetOnAxis
from concourse.masks import make_identity, make_upper_triangular

F32 = mybir.dt.float32
F32R = mybir.dt.float32r
I32 = mybir.dt.int32
AX = mybir.AxisListType
OP = mybir.AluOpType
ACT = mybir.ActivationFunctionType

P = 128
NTOK = 4096
D = 1024
F = 4096
E = 8
NCORES = 8
SLICE = NTOK // NCORES          # router tokens per core
C = 640                         # expert token capacity per core
CHUNKS = [(0, 0, 384), (1, 384, 256)]  # (psum bank, slot offset, width);
                                       # each >=256 wide for f32r full rate
NI = NTOK // P                  # 32 tokens per partition (t = p*32 + i)
ND = D // P                     # 8
NF = F // P                     # 32
NST = C // P                    # 5 slot tiles
OOB_PAD = 100000.0              # slot value for tokens not on this expert


def build_nc(act_fn=None):
    nc = bacc.Bacc("TRN2", target_bir_lowering=False, debug=False,
                   num_devices=NCORES)

    x = nc.dram_tensor("x", [NTOK, D], F32, kind="ExternalInput").ap()
    xtt = nc.dram_tensor("xtt", [NTOK // 256, P, ND, 256], F32,
                         kind="ExternalInput").ap()
    rwt = nc.dram_tensor("rwt", [D, E], F32, kind="ExternalInput").ap()
    rb = nc.dram_tensor("rb", [1, E], F32, kind="ExternalInput").ap()
    rbt = nc.dram_tensor("rbt", [E, 1], F32, kind="ExternalInput").ap()
    w1t = nc.dram_tensor("w1t", [NF, P, ND, P], F32, kind="ExternalInput").ap()
    b1 = nc.dram_tensor("b1", [F, 1], F32, kind="ExternalInput").ap()
    w2t = nc.dram_tensor("w2t", [ND, P, NF, P], F32, kind="ExternalInput").ap()
    b2 = nc.dram_tensor("b2", [D, 1], F32, kind="ExternalInput").ap()
    sel = nc.dram_tensor("sel", [1, 1, E], F32, kind="ExternalInput").ap()

    out = nc.dram_tensor("out", [NTOK, D], F32, kind="ExternalOutput").ap()
    lgout = nc.dram_tensor("lgout", [NTOK, E], F32, kind="ExternalOutput").ap()
    prout = nc.dram_tensor("prout", [NTOK, E], F32, kind="ExternalOutput").ap()

    with tile.TileContext(nc) as tc:
        build_kernel(tc, x, xtt, rwt, rb, rbt, w1t, b1, w2t, b2, sel,
                     out, lgout, prout, act_fn=act_fn or ACT.Gelu)
    nc.compile()
    return nc


def build_kernel(tc, x, xtt, rwt, rb, rbt, w1t, b1, w2t, b2, sel,
                 out, lgout, prout, act_fn=ACT.Gelu):
    nc = tc.nc
    with (
        tc.tile_pool(name="const", bufs=1) as const,
        tc.tile_pool(name="small", bufs=2) as small,
        tc.tile_pool(name="routr", bufs=4) as routr,
        tc.tile_pool(name="xg", bufs=1) as xg_pool,
        tc.tile_pool(name="hT", bufs=1) as hT_pool,
        tc.tile_pool(name="w1p", bufs=3) as w1p,
        tc.tile_pool(name="w2p", bufs=2) as w2p,
        tc.tile_pool(name="yy", bufs=1) as y_pool,
        tc.tile_pool(name="psA", bufs=2, space="PSUM") as psA,      # 2x2 banks
        tc.tile_pool(name="psB", bufs=1, space="PSUM") as psB,      # 1x2 banks
        tc.tile_pool(name="psT", bufs=2, space="PSUM") as psT,      # 2x1 banks
        tc.tile_pool(name="dram", bufs=1, space="DRAM") as dram,
    ):
        # ---- constants ----
        ident = const.tile([P, P], F32)
        make_identity(nc, ident[:])
        ut = const.tile([P, P], F32)                  # strictly-upper ones
        make_upper_triangular(nc, ut[:], val=1.0, diag=False)
        rwt_sb = const.tile([P, ND, E], F32)
        nc.sync.dma_start(out=rwt_sb[:], in_=rwt.rearrange("(dt p) e -> p dt e", p=P))
        rb_sb = const.tile([P, E], F32)
        nc.sync.dma_start(out=rb_sb[:], in_=rb.to_broadcast([P, E]))
        rbt_sb = const.tile([E, 1], F32)
        nc.sync.dma_start(out=rbt_sb[:], in_=rbt[:, :])
        sel_sb = const.tile([P, 1, E], F32)
        nc.sync.dma_start(out=sel_sb[:], in_=sel.to_broadcast([P, 1, E]))
        b1_sb = const.tile([P, NF], F32)
        nc.sync.dma_start(out=b1_sb[:], in_=b1.rearrange("(ft p) one -> p (ft one)", p=P))
        b2_sb = const.tile([P, ND], F32)
        nc.sync.dma_start(out=b2_sb[:], in_=b2.rearrange("(dt p) one -> p (dt one)", p=P))
        zeros32 = const.tile([P, NI], F32)
        nc.vector.memset(zeros32[:], 0.0)
        jrow = const.tile([P, C], F32)

        with (
            tc.tile_pool(name="xtp", bufs=2) as xt_pool,
            tc.tile_pool(name="xgp", bufs=2) as xgrow_pool,
        ):
            # ---- phase 1: full router on every core (t = p + 128*i) ----
            # logitsT[8, tok] = rwt.T @ xt, router weights stationary.
            jrow_i = small.tile([P, C], I32, tag="mi", bufs=3)
            nc.gpsimd.iota(jrow_i[:], pattern=[[1, C]], base=0,
                           channel_multiplier=0)
            nc.vector.tensor_copy(out=jrow[:], in_=jrow_i[:])
            mask = small.tile([P, NI], F32, tag="mask")
            TCH = 256        # tokens per router matmul chunk
            for tch in range(NTOK // TCH):
                xt_sb = xt_pool.tile([P, ND, TCH], F32, tag="xt")
                nc.sync.dma_start(out=xt_sb[:], in_=xtt[tch])
                pslt = psT.tile([E, TCH], F32, space="PSUM", tag="pst")
                for dt_ in range(ND):
                    nc.tensor.matmul(pslt[:], lhsT=rwt_sb[:, dt_, :],
                                     rhs=xt_sb[:, dt_, :],
                                     start=(dt_ == 0), stop=(dt_ == ND - 1))
                lgT = routr.tile([E, TCH], F32, tag="lgT")
                nc.vector.tensor_scalar_add(lgT[:], pslt[:], rbt_sb[:, :])
                for j in range(TCH // P):
                    i = tch * (TCH // P) + j
                    pslg = psT.tile([P, E], F32, space="PSUM", tag="pst")
                    nc.tensor.transpose(out=pslg[:],
                                        in_=lgT[:, j * P:(j + 1) * P],
                                        identity=ident[:E, :E])
                    lg = routr.tile([P, E], F32, tag="lg")
                    nc.vector.tensor_copy(out=lg[:], in_=pslg[:])
                    nc.sync.dma_start(out=lgout[i * P:(i + 1) * P, :], in_=lg[:])
                    # own-expert mask column (critical path)
                    mx = routr.tile([P, 1], F32, tag="mx")
                    nc.vector.tensor_reduce(out=mx[:], in_=lg[:], axis=AX.X,
                                            op=OP.max)
                    oh = routr.tile([P, E], F32, tag="oh")
                    nc.vector.tensor_tensor(out=oh[:], in0=lg[:],
                                            in1=mx[:].to_broadcast([P, E]),
                                            op=OP.is_equal)
                    ohs = routr.tile([P, E], F32, tag="ohs")
                    nc.vector.tensor_tensor(out=ohs[:], in0=oh[:],
                                            in1=sel_sb[:, 0, :], op=OP.mult)
                    nc.vector.tensor_reduce(out=mask[:, i:i + 1], in_=ohs[:],
                                            axis=AX.X, op=OP.add)
                    # softmax (off critical path)
                    negmx = routr.tile([P, 1], F32, tag="negmx")
                    nc.vector.tensor_reduce(out=negmx[:], in_=lg[:], axis=AX.X,
                                            op=OP.max, negate=True)
                    ex = routr.tile([P, E], F32, tag="ex")
                    nc.scalar.activation(out=ex[:], in_=lg[:], func=ACT.Exp,
                                         bias=negmx[:, :])
                    sm = routr.tile([P, 1], F32, tag="sm")
                    nc.vector.tensor_reduce(out=sm[:], in_=ex[:], axis=AX.X,
                                            op=OP.add)
                    rcp = routr.tile([P, 1], F32, tag="rcp")
                    nc.vector.reciprocal(out=rcp[:], in_=sm[:])
                    pr = routr.tile([P, E], F32, tag="pr")
                    nc.vector.tensor_scalar_mul(pr[:], ex[:], rcp[:, :])
                    nc.sync.dma_start(out=prout[i * P:(i + 1) * P, :], in_=pr[:])

            # ---- phase 3: compaction (matmul-based, no DRAM roundtrip) ----
            cum = small.tile([P, NI], F32, tag="cum")
            nc.vector.tensor_tensor_scan(out=cum[:], data0=mask[:], data1=zeros32[:],
                                         initial=0.0, op0=OP.add, op1=OP.max)
            pspp = psT.tile([P, 1], F32, space="PSUM", tag="pst")
            nc.tensor.matmul(pspp[:], lhsT=ut[:], rhs=cum[:, NI - 1:NI],
                             start=True, stop=True)
            pp = small.tile([P, 1], F32, tag="pp")
            nc.vector.tensor_copy(out=pp[:], in_=pspp[:])
            slot = small.tile([P, NI], F32, tag="slot")
            nc.vector.tensor_tensor(out=slot[:], in0=cum[:], in1=mask[:],
                                    op=OP.subtract)
            nc.vector.tensor_tensor(out=slot[:], in0=slot[:],
                                    in1=pp[:].to_broadcast([P, NI]), op=OP.add)
            # slots for masked-out tokens -> huge (matches no j, so dropped)
            nc.vector.tensor_scalar_add(slot[:], slot[:], -OOB_PAD)
            nc.vector.tensor_tensor(out=slot[:], in0=slot[:], in1=mask[:],
                                    op=OP.mult)
            nc.vector.tensor_scalar_add(slot[:], slot[:], OOB_PAD)
            # rhs pairs [tokid, 1.0] per token column
            tokid = small.tile([P, NI], I32, tag="tokid")
            nc.gpsimd.iota(tokid[:], pattern=[[P, NI]], base=0,
                           channel_multiplier=1)
            tok2 = small.tile([P, NI, 2], F32, tag="tok2")
            nc.vector.memset(tok2[:], 1.0)
            nc.vector.tensor_copy(out=tok2[:, :, 0], in_=tokid[:])
            # gidx[j] = sum_t tokid[t]*[slot(t)==j]; coverage in column 1
            gsum = small.tile([P, NST, 2], F32, tag="gsum")
            nc.vector.memset(gsum[:], 0.0)
            for i in range(NI):
                mi = small.tile([P, C], F32, tag="mi", bufs=3)
                nc.vector.tensor_scalar(out=mi[:], in0=jrow[:],
                                        scalar1=slot[:, i:i + 1], scalar2=None,
                                        op0=OP.is_equal)
                psg = psT.tile([P, NST, 2], F32, space="PSUM", tag="pst")
                for st in range(NST):
                    nc.tensor.matmul(psg[:, st, :],
                                     lhsT=mi[:, st * P:(st + 1) * P],
                                     rhs=tok2[:, i, :],
                                     start=True, stop=True)
                nc.vector.tensor_tensor(out=gsum[:], in0=gsum[:], in1=psg[:],
                                        op=OP.add)
            # pad slots (coverage 0) -> NTOK (gather clamps; scatter skips)
            gf = small.tile([P, NST], F32, tag="gf")
            nc.vector.tensor_scalar(out=gf[:], in0=gsum[:, :, 1], scalar1=-float(NTOK),
                                    scalar2=float(NTOK), op0=OP.mult, op1=OP.add)
            nc.vector.tensor_tensor(out=gf[:], in0=gf[:], in1=gsum[:, :, 0],
                                    op=OP.add)
            gidx = small.tile([P, NST], I32, tag="gidx")
            nc.vector.tensor_copy(out=gidx[:], in_=gf[:])
            nc.vector.tensor_scalar_min(gf[:], gf[:], float(NTOK - 1))
            gci = small.tile([P, NST], I32, tag="gci")
            nc.vector.tensor_copy(out=gci[:], in_=gf[:])

            # ---- phase 4: gather x rows, transpose to [d, slot] ----
            xgT = xg_pool.tile([P, ND, C], F32R)
            for st in range(NST):
                xg = xgrow_pool.tile([P, D], F32, tag="xg")
                nc.gpsimd.indirect_dma_start(
                    out=xg[:], out_offset=None,
                    in_=x[:, :],
                    in_offset=IndirectOffsetOnAxis(ap=gci[:, st:st + 1], axis=0),
                )
                for dt_ in range(ND):
                    pst = psT.tile([P, P], F32, space="PSUM", tag="pst")
                    nc.tensor.transpose(out=pst[:],
                                        in_=xg[:, dt_ * P:(dt_ + 1) * P],
                                        identity=ident[:])
                    nc.vector.tensor_copy(out=xgT[:, dt_, st * P:(st + 1) * P],
                                          in_=pst[:])

        # ---- phase 5: layer 1 (hT[f, slot] = gelu(W1.T @ xgT + b1)) ----
        hT = hT_pool.tile([P, NF, C], F32R)
        for ft in range(NF):
            w1row = w1p.tile([P, ND, P], F32R, tag="w1")
            nc.sync.dma_start(out=w1row[:], in_=w1t[ft].bitcast(F32R))
            psh = psA.tile([P, 2, 512], F32, space="PSUM")
            for dt_ in range(ND):
                for (cb, c0, cn) in CHUNKS:
                    nc.tensor.matmul(
                        psh[:, cb, 0:cn],
                        lhsT=w1row[:, dt_, :],
                        rhs=xgT[:, dt_, c0:c0 + cn],
                        start=(dt_ == 0), stop=(dt_ == ND - 1),
                    )
            for (cb, c0, cn) in CHUNKS:
                nc.scalar.activation(out=hT[:, ft, c0:c0 + cn], in_=psh[:, cb, 0:cn],
                                     func=act_fn, bias=b1_sb[:, ft:ft + 1])

        # ---- phase 6: layer 2 (yT[d, slot] = W2.T @ hT + b2), transpose, scatter
        y_tiles = [y_pool.tile([P, D], F32, tag=f"y{st}", name=f"y{st}")
                   for st in range(NST)]
        NFH = NF // 2
        for dt_ in range(ND):
            psy = psB.tile([P, 2, 512], F32, space="PSUM")
            for half in range(2):
                w2row = w2p.tile([P, NFH, P], F32R, tag="w2")
                nc.sync.dma_start(
                    out=w2row[:],
                    in_=w2t[dt_, :, half * NFH:(half + 1) * NFH, :].bitcast(F32R))
                for fh in range(NFH):
                    ft = half * NFH + fh
                    for (cb, c0, cn) in CHUNKS:
                        nc.tensor.matmul(
                            psy[:, cb, 0:cn],
                            lhsT=w2row[:, fh, :],
                            rhs=hT[:, ft, c0:c0 + cn],
                            start=(ft == 0), stop=(ft == NF - 1),
                        )
            yT = small.tile([P, C], F32, tag="yT")
            for (cb, c0, cn) in CHUNKS:
                nc.vector.tensor_scalar_add(yT[:, c0:c0 + cn], psy[:, cb, 0:cn],
                                            b2_sb[:, dt_:dt_ + 1])
            for st in range(NST):
                pst = psT.tile([P, P], F32, space="PSUM", tag="pst")
                nc.tensor.transpose(out=pst[:], in_=yT[:, st * P:(st + 1) * P],
                                    identity=ident[:])
                nc.vector.tensor_copy(out=y_tiles[st][:, dt_ * P:(dt_ + 1) * P],
                                      in_=pst[:])
        for st in range(NST):
            nc.gpsimd.indirect_dma_start(
                out=out[:, :],
                out_offset=IndirectOffsetOnAxis(ap=gidx[:, st:st + 1], axis=0),
                in_=y_tiles[st][:],
                in_offset=None,
                bounds_check=NTOK - 1, oob_is_err=False,
            )


_NC_CACHE = None


def _get_nc():
    global _NC_CACHE
    if _NC_CACHE is None:
        _NC_CACHE = build_nc()
    return _NC_CACHE


def make_in_maps(x, router_W, router_b, W1, b1, W2, b2):
    xf = np.ascontiguousarray(np.asarray(x).reshape(NTOK, D).astype(np.float32))
    # xtt[tch, p, dt, t] = x[tch*512 + t, dt*128 + p]
    xtt = np.ascontiguousarray(
        xf.reshape(NTOK // 256, 256, ND, P).transpose(0, 3, 2, 1))
    rwt = np.ascontiguousarray(np.asarray(router_W).astype(np.float32).T)  # [D, E]
    rbv = np.asarray(router_b).astype(np.float32).reshape(1, E)
    in_maps = []
    for c in range(NCORES):
        w1c = np.asarray(W1[c]).astype(np.float32)                  # [D, F]
        w2c = np.asarray(W2[c]).astype(np.float32)                  # [F, D]
        # w1t[ft, p, dt, j] = W1[dt*128+p, ft*128+j]
        w1tile = np.ascontiguousarray(
            w1c.reshape(ND, P, NF, P).transpose(2, 1, 0, 3))
        # w2t[dt, p, ft, j] = W2[ft*128+p, dt*128+j]
        w2tile = np.ascontiguousarray(
            w2c.reshape(NF, P, ND, P).transpose(2, 1, 0, 3))
        selv = np.zeros((1, 1, E), np.float32)
        selv[0, 0, c] = 1.0
        in_maps.append({
            "x": xf,
            "xtt": xtt,
            "rwt": rwt,
            "rb": rbv,
            "rbt": rbv.reshape(E, 1).copy(),
            "w1t": w1tile,
            "b1": np.asarray(b1[c]).astype(np.float32).reshape(F, 1),
            "w2t": w2tile,
            "b2": np.asarray(b2[c]).astype(np.float32).reshape(D, 1),
            "sel": selv,
        })
    return in_maps


def combine_results(results):
    out = np.zeros((NTOK, D), np.float32)
    for r in results:
        out += r["out"]
    logits = results[0]["lgout"]
    probs = results[0]["prout"]
    return (out.reshape(2, 2048, D),
            probs.reshape(2, 2048, E),
            logits.reshape(2, 2048, E))


def kernel(x, router_W, router_b, W1, b1, W2, b2):
    from concourse.bass_utils import run_bass_kernel_spmd
    nc = _get_nc()
    in_maps = make_in_maps(x, router_W, router_b, W1, b1, W2, b2)
    res = run_bass_kernel_spmd(nc, in_maps, core_ids=list(range(NCORES)))
    return combine_results(res.results)


# revision 32
# speedup vs baseline: 1.4182x; 1.0109x over previous
"""Expert-parallel top-1 MoE FFN kernel for 8 Trainium2 NeuronCores.

Problem: x[2,2048,1024] routed (top-1 of softmax(x @ rW.T + rb)) through one of
E=8 expert FFNs (Linear(1024,4096) -> gelu -> Linear(4096,1024)).
Returns (expert_outputs, router_probs, router_logits).

Strategy (one expert per core, fully independent cores — no collectives;
an NRT start barrier + AllGather measured ~84us, more than recomputing the
router locally):
  - every core computes the full router (all 4096 tokens) from a
    host-transposed copy of x (xt = x.T, layout prep in the host wrapper):
    64 dense fp32 matmuls with the tiny router weights stationary produce
    logitsT[8, tok]; small PE transposes flip each 128-token tile back to
    [tok, 8] for softmax/argmax (t = p + 128*i layout)
  - compacts its expert's token ids with an on-device prefix-sum
    (tensor_tensor_scan + triangular matmul) and per-column indirect-DMA
    scatters into a DRAM perm table (pad slots -> OOB, skipped)
  - indirect-DMA gather of those <=C=640 token rows, FFN with float32r
    matmuls; W1/W2 are host-retiled so they stream from HBM exactly once in
    512KB/1MB contiguous chunks; indirect-DMA scatter of outputs
  - host sums the 8 disjoint per-core outputs; router outputs from core 0
"""

import numpy as np

import concourse.bass as bass
import concourse.tile as tile
from concourse import bacc, mybir
from concourse.bass import IndirectOffsetOnAxis
from concourse.masks import make_identity, make_upper_triangular

F32 = mybir.dt.float32
F32R = mybir.dt.float32r
I32 = mybir.dt.int32
AX = mybir.AxisListType
OP = mybir.AluOpType
ACT = mybir.ActivationFunctionType

P = 128
NTOK = 4096
D = 1024
F = 4096
E = 8
NCORES = 8
SLICE = NTOK // NCORES          # router tokens per core
C = 640                         # expert token capacity per core
CHUNKS = [(0, 0, 384), (1, 384, 256)]  # (psum bank, slot offset, width);
                                       # each >=256 wide for f32r full rate
NI = NTOK // P                  # 32 tokens per partition (t = p*32 + i)
ND = D // P                     # 8
NF = F // P                     # 32
NST = C // P                    # 5 slot tiles
OOB_PAD = 100000.0              # slot value for tokens not on this expert


def build_nc(act_fn=None):
    nc = bacc.Bacc("TRN2", target_bir_lowering=False, debug=False,
                   num_devices=NCORES)

    x = nc.dram_tensor("x", [NTOK, D], F32, kind="ExternalInput").ap()
    xtt = nc.dram_tensor("xtt", [NTOK // 512, P, ND, 512], F32,
                         kind="ExternalInput").ap()
    rwt = nc.dram_tensor("rwt", [D, E], F32, kind="ExternalInput").ap()
    rb = nc.dram_tensor("rb", [1, E], F32, kind="ExternalInput").ap()
    rbt = nc.dram_tensor("rbt", [E, 1], F32, kind="ExternalInput").ap()
    w1t = nc.dram_tensor("w1t", [NF, P, ND, P], F32, kind="ExternalInput").ap()
    b1 = nc.dram_tensor("b1", [F, 1], F32, kind="ExternalInput").ap()
    w2t = nc.dram_tensor("w2t", [ND, P, NF, P], F32, kind="ExternalInput").ap()
    b2 = nc.dram_tensor("b2", [D, 1], F32, kind="ExternalInput").ap()
    sel = nc.dram_tensor("sel", [1, 1, E], F32, kind="ExternalInput").ap()

    out = nc.dram_tensor("out", [NTOK, D], F32, kind="ExternalOutput").ap()
    lgout = nc.dram_tensor("lgout", [NTOK, E], F32, kind="ExternalOutput").ap()
    prout = nc.dram_tensor("prout", [NTOK, E], F32, kind="ExternalOutput").ap()

    with tile.TileContext(nc) as tc:
        build_kernel(tc, x, xtt, rwt, rb, rbt, w1t, b1, w2t, b2, sel,
                     out, lgout, prout, act_fn=act_fn or ACT.Gelu)
    nc.compile()
    return nc


def build_kernel(tc, x, xtt, rwt, rb, rbt, w1t, b1, w2t, b2, sel,
                 out, lgout, prout, act_fn=ACT.Gelu):
    nc = tc.nc
    with (
        tc.tile_pool(name="const", bufs=1) as const,
        tc.tile_pool(name="small", bufs=2) as small,
        tc.tile_pool(name="routr", bufs=4) as routr,
        tc.tile_pool(name="xg", bufs=1) as xg_pool,
        tc.tile_pool(name="hT", bufs=1) as hT_pool,
        tc.tile_pool(name="psA", bufs=2, space="PSUM") as psA,      # 2x2 banks
        tc.tile_pool(name="psB", bufs=1, space="PSUM") as psB,      # 1x2 banks
        tc.tile_pool(name="psT", bufs=2, space="PSUM") as psT,      # 2x1 banks
        tc.tile_pool(name="dram", bufs=1, space="DRAM") as dram,
    ):
        # ---- constants ----
        ident = const.tile([P, P], F32)
        make_identity(nc, ident[:])
        ut = const.tile([P, P], F32)                  # strictly-upper ones
        make_upper_triangular(nc, ut[:], val=1.0, diag=False)
        rwt_sb = const.tile([P, ND, E], F32)
        nc.sync.dma_start(out=rwt_sb[:], in_=rwt.rearrange("(dt p) e -> p dt e", p=P))
        rb_sb = const.tile([P, E], F32)
        nc.sync.dma_start(out=rb_sb[:], in_=rb.to_broadcast([P, E]))
        rbt_sb = const.tile([E, 1], F32)
        nc.sync.dma_start(out=rbt_sb[:], in_=rbt[:, :])
        sel_sb = const.tile([P, 1, E], F32)
        nc.sync.dma_start(out=sel_sb[:], in_=sel.to_broadcast([P, 1, E]))
        b1_sb = const.tile([P, NF], F32)
        nc.sync.dma_start(out=b1_sb[:], in_=b1.rearrange("(ft p) one -> p (ft one)", p=P))
        b2_sb = const.tile([P, ND], F32)
        nc.sync.dma_start(out=b2_sb[:], in_=b2.rearrange("(dt p) one -> p (dt one)", p=P))
        zeros32 = const.tile([P, NI], F32)
        nc.vector.memset(zeros32[:], 0.0)
        jrow = const.tile([P, C], F32)

        with (
            tc.tile_pool(name="xtp", bufs=3) as xt_pool,
            tc.tile_pool(name="xgp", bufs=2) as xgrow_pool,
        ):
            # ---- phase 1: full router on every core (t = p + 128*i) ----
            # logitsT[8, tok] = rwt.T @ xt, router weights stationary.
            jrow_i = small.tile([P, C], I32, tag="mi", bufs=3)
            nc.gpsimd.iota(jrow_i[:], pattern=[[1, C]], base=0,
                           channel_multiplier=0)
            nc.vector.tensor_copy(out=jrow[:], in_=jrow_i[:])
            mask = small.tile([P, NI], F32, tag="mask")
            TCH = 512        # tokens per router matmul chunk
            for tch in range(NTOK // TCH):
                xt_sb = xt_pool.tile([P, ND, TCH], F32, tag="xt")
                nc.sync.dma_start(out=xt_sb[:], in_=xtt[tch])
                pslt = psT.tile([E, TCH], F32, space="PSUM", tag="pst")
                for dt_ in range(ND):
                    nc.tensor.matmul(pslt[:], lhsT=rwt_sb[:, dt_, :],
                                     rhs=xt_sb[:, dt_, :],
                                     start=(dt_ == 0), stop=(dt_ == ND - 1))
                lgT = routr.tile([E, TCH], F32, tag="lgT")
                nc.vector.tensor_scalar_add(lgT[:], pslt[:], rbt_sb[:, :])
                for j in range(TCH // P):
                    i = tch * (TCH // P) + j
                    pslg = psT.tile([P, E], F32, space="PSUM", tag="pst")
                    nc.tensor.transpose(out=pslg[:],
                                        in_=lgT[:, j * P:(j + 1) * P],
                                        identity=ident[:E, :E])
                    lg = routr.tile([P, E], F32, tag="lg")
                    nc.vector.tensor_copy(out=lg[:], in_=pslg[:])
                    nc.sync.dma_start(out=lgout[i * P:(i + 1) * P, :], in_=lg[:])
                    # own-expert mask column (critical path)
                    mx = routr.tile([P, 1], F32, tag="mx")
                    nc.vector.tensor_reduce(out=mx[:], in_=lg[:], axis=AX.X,
                                            op=OP.max)
                    oh = routr.tile([P, E], F32, tag="oh")
                    nc.vector.tensor_tensor(out=oh[:], in0=lg[:],
                                            in1=mx[:].to_broadcast([P, E]),
                                            op=OP.is_equal)
                    ohs = routr.tile([P, E], F32, tag="ohs")
                    nc.vector.tensor_tensor(out=ohs[:], in0=oh[:],
                                            in1=sel_sb[:, 0, :], op=OP.mult)
                    nc.vector.tensor_reduce(out=mask[:, i:i + 1], in_=ohs[:],
                                            axis=AX.X, op=OP.add)
                    # softmax (off critical path)
                    negmx = routr.tile([P, 1], F32, tag="negmx")
                    nc.vector.tensor_reduce(out=negmx[:], in_=lg[:], axis=AX.X,
                                            op=OP.max, negate=True)
                    ex = routr.tile([P, E], F32, tag="ex")
                    nc.scalar.activation(out=ex[:], in_=lg[:], func=ACT.Exp,
                                         bias=negmx[:, :])
                    sm = routr.tile([P, 1], F32, tag="sm")
                    nc.vector.tensor_reduce(out=sm[:], in_=ex[:], axis=AX.X,
                                            op=OP.add)
                    rcp = routr.tile([P, 1], F32, tag="rcp")
                    nc.vector.reciprocal(out=rcp[:], in_=sm[:])
                    pr = routr.tile([P, E], F32, tag="pr")
                    nc.vector.tensor_scalar_mul(pr[:], ex[:], rcp[:, :])
                    nc.sync.dma_start(out=prout[i * P:(i + 1) * P, :], in_=pr[:])

            # ---- phase 3: compaction (matmul-based, no DRAM roundtrip) ----
            cum = small.tile([P, NI], F32, tag="cum")
            nc.vector.tensor_tensor_scan(out=cum[:], data0=mask[:], data1=zeros32[:],
                                         initial=0.0, op0=OP.add, op1=OP.max)
            pspp = psT.tile([P, 1], F32, space="PSUM", tag="pst")
            nc.tensor.matmul(pspp[:], lhsT=ut[:], rhs=cum[:, NI - 1:NI],
                             start=True, stop=True)
            pp = small.tile([P, 1], F32, tag="pp")
            nc.vector.tensor_copy(out=pp[:], in_=pspp[:])
            slot = small.tile([P, NI], F32, tag="slot")
            nc.vector.tensor_tensor(out=slot[:], in0=cum[:], in1=mask[:],
                                    op=OP.subtract)
            nc.vector.tensor_tensor(out=slot[:], in0=slot[:],
                                    in1=pp[:].to_broadcast([P, NI]), op=OP.add)
            # slots for masked-out tokens -> huge (matches no j, so dropped)
            nc.vector.tensor_scalar_add(slot[:], slot[:], -OOB_PAD)
            nc.vector.tensor_tensor(out=slot[:], in0=slot[:], in1=mask[:],
                                    op=OP.mult)
            nc.vector.tensor_scalar_add(slot[:], slot[:], OOB_PAD)
            # rhs pairs [tokid, 1.0] per token column
            tokid = small.tile([P, NI], I32, tag="tokid")
            nc.gpsimd.iota(tokid[:], pattern=[[P, NI]], base=0,
                           channel_multiplier=1)
            tok2 = small.tile([P, NI, 2], F32, tag="tok2")
            nc.vector.memset(tok2[:], 1.0)
            nc.vector.tensor_copy(out=tok2[:, :, 0], in_=tokid[:])
            # gidx[j] = sum_t tokid[t]*[slot(t)==j]; coverage in column 1.
            # Batched 4 token-columns per DVE op to amortize op overheads.
            NB = 4
            gsum4 = small.tile([P, NB, NST, 2], F32, tag="gsum4")
            nc.vector.memset(gsum4[:], 0.0)
            for g in range(NI // NB):
                mi4 = small.tile([P, NB, C], F32, tag="mi4", bufs=2)
                slot4 = slot[:, g * NB:(g + 1) * NB]
                slot4b = bass.AP(tensor=slot4.tensor, offset=slot4.offset,
                                 ap=[*slot4.ap, [0, C]])
                jrow4 = bass.AP(tensor=jrow[:].tensor, offset=jrow[:].offset,
                                ap=[jrow[:].ap[0], [0, NB], jrow[:].ap[1]])
                nc.vector.tensor_tensor(out=mi4[:], in0=jrow4, in1=slot4b,
                                        op=OP.is_equal)
                psg = psT.tile([P, NB, NST, 2], F32, space="PSUM", tag="pst")
                for sub in range(NB):
                    for st in range(NST):
                        nc.tensor.matmul(psg[:, sub, st, :],
                                         lhsT=mi4[:, sub, st * P:(st + 1) * P],
                                         rhs=tok2[:, g * NB + sub, :],
                                         start=True, stop=True)
                nc.vector.tensor_tensor(out=gsum4[:], in0=gsum4[:], in1=psg[:],
                                        op=OP.add)
            gsum = small.tile([P, NST, 2], F32, tag="gsum")
            nc.vector.tensor_tensor(out=gsum[:], in0=gsum4[:, 0], in1=gsum4[:, 1],
                                    op=OP.add)
            nc.vector.tensor_tensor(out=gsum[:], in0=gsum[:], in1=gsum4[:, 2],
                                    op=OP.add)
            nc.vector.tensor_tensor(out=gsum[:], in0=gsum[:], in1=gsum4[:, 3],
                                    op=OP.add)
            # pad slots (coverage 0) -> NTOK (gather clamps; scatter skips)
            gf = small.tile([P, NST], F32, tag="gf")
            nc.vector.tensor_scalar(out=gf[:], in0=gsum[:, :, 1], scalar1=-float(NTOK),
                                    scalar2=float(NTOK), op0=OP.mult, op1=OP.add)
            nc.vector.tensor_tensor(out=gf[:], in0=gf[:], in1=gsum[:, :, 0],
                                    op=OP.add)
            gidx = small.tile([P, NST], I32, tag="gidx")
            nc.vector.tensor_copy(out=gidx[:], in_=gf[:])
            nc.vector.tensor_scalar_min(gf[:], gf[:], float(NTOK - 1))
            gci = small.tile([P, NST], I32, tag="gci")
            nc.vector.tensor_copy(out=gci[:], in_=gf[:])

            # ---- phase 4: gather x rows, transpose to [d, slot] ----
            xgT = xg_pool.tile([P, ND, C], F32R)
            for st in range(NST):
                xg = xgrow_pool.tile([P, D], F32, tag="xg")
                nc.gpsimd.indirect_dma_start(
                    out=xg[:], out_offset=None,
                    in_=x[:, :],
                    in_offset=IndirectOffsetOnAxis(ap=gci[:, st:st + 1], axis=0),
                )
                for dt_ in range(ND):
                    pst = psT.tile([P, P], F32, space="PSUM", tag="pst")
                    nc.tensor.transpose(out=pst[:],
                                        in_=xg[:, dt_ * P:(dt_ + 1) * P],
                                        identity=ident[:])
                    nc.vector.tensor_copy(out=xgT[:, dt_, st * P:(st + 1) * P],
                                          in_=pst[:])

        # ---- phase 5: layer 1 (hT[f, slot] = gelu(W1.T @ xgT + b1)) ----
        ffn_pools = tc.tile_pool(name="w1p", bufs=3)
        w1p = ffn_pools.__enter__()
        w2p_cm = tc.tile_pool(name="w2p", bufs=2)
        w2p = w2p_cm.__enter__()
        yy_cm = tc.tile_pool(name="yy", bufs=1)
        y_pool = yy_cm.__enter__()
        hT = hT_pool.tile([P, NF, C], F32R)
        for ft in range(NF):
            w1row = w1p.tile([P, ND, P], F32R, tag="w1")
            nc.sync.dma_start(out=w1row[:], in_=w1t[ft].bitcast(F32R))
            psh = psA.tile([P, 2, 512], F32, space="PSUM")
            for dt_ in range(ND):
                for (cb, c0, cn) in CHUNKS:
                    nc.tensor.matmul(
                        psh[:, cb, 0:cn],
                        lhsT=w1row[:, dt_, :],
                        rhs=xgT[:, dt_, c0:c0 + cn],
                        start=(dt_ == 0), stop=(dt_ == ND - 1),
                    )
            for (cb, c0, cn) in CHUNKS:
                nc.scalar.activation(out=hT[:, ft, c0:c0 + cn], in_=psh[:, cb, 0:cn],
                                     func=act_fn, bias=b1_sb[:, ft:ft + 1])

        # ---- phase 6: layer 2 (yT[d, slot] = W2.T @ hT + b2), transpose, scatter
        y_tiles = [y_pool.tile([P, D], F32, tag=f"y{st}", name=f"y{st}")
                   for st in range(NST)]
        NFH = NF // 2
        for dt_ in range(ND):
            psy = psB.tile([P, 2, 512], F32, space="PSUM")
            for half in range(2):
                w2row = w2p.tile([P, NFH, P], F32R, tag="w2")
                nc.sync.dma_start(
                    out=w2row[:],
                    in_=w2t[dt_, :, half * NFH:(half + 1) * NFH, :].bitcast(F32R))
                for fh in range(NFH):
                    ft = half * NFH + fh
                    for (cb, c0, cn) in CHUNKS:
                        nc.tensor.matmul(
                            psy[:, cb, 0:cn],
                            lhsT=w2row[:, fh, :],
                            rhs=hT[:, ft, c0:c0 + cn],
                            start=(ft == 0), stop=(ft == NF - 1),
                        )
            yT = small.tile([P, C], F32, tag="yT")
            for (cb, c0, cn) in CHUNKS:
                nc.vector.tensor_scalar_add(yT[:, c0:c0 + cn], psy[:, cb, 0:cn],
                                            b2_sb[:, dt_:dt_ + 1])
            for st in range(NST):
                pst = psT.tile([P, P], F32, space="PSUM", tag="pst")
                nc.tensor.transpose(out=pst[:], in_=yT[:, st * P:(st + 1) * P],
                                    identity=ident[:])
                nc.vector.tensor_copy(out=y_tiles[st][:, dt_ * P:(dt_ + 1) * P],
                                      in_=pst[:])
        for st in range(NST):
            nc.gpsimd.indirect_dma_start(
                out=out[:, :],
                out_offset=IndirectOffsetOnAxis(ap=gidx[:, st:st + 1], axis=0),
                in_=y_tiles[st][:],
                in_offset=None,
                bounds_check=NTOK - 1, oob_is_err=False,
            )
        yy_cm.__exit__(None, None, None)
        w2p_cm.__exit__(None, None, None)
        ffn_pools.__exit__(None, None, None)


_NC_CACHE = None


def _get_nc():
    global _NC_CACHE
    if _NC_CACHE is None:
        _NC_CACHE = build_nc()
    return _NC_CACHE


def make_in_maps(x, router_W, router_b, W1, b1, W2, b2):
    xf = np.ascontiguousarray(np.asarray(x).reshape(NTOK, D).astype(np.float32))
    # xtt[tch, p, dt, t] = x[tch*512 + t, dt*128 + p]
    xtt = np.ascontiguousarray(
        xf.reshape(NTOK // 512, 512, ND, P).transpose(0, 3, 2, 1))
    rwt = np.ascontiguousarray(np.asarray(router_W).astype(np.float32).T)  # [D, E]
    rbv = np.asarray(router_b).astype(np.float32).reshape(1, E)
    in_maps = []
    for c in range(NCORES):
        w1c = np.asarray(W1[c]).astype(np.float32)                  # [D, F]
        w2c = np.asarray(W2[c]).astype(np.float32)                  # [F, D]
        # w1t[ft, p, dt, j] = W1[dt*128+p, ft*128+j]
        w1tile = np.ascontiguousarray(
            w1c.reshape(ND, P, NF, P).transpose(2, 1, 0, 3))
        # w2t[dt, p, ft, j] = W2[ft*128+p, dt*128+j]
        w2tile = np.ascontiguousarray(
            w2c.reshape(NF, P, ND, P).transpose(2, 1, 0, 3))
        selv = np.zeros((1, 1, E), np.float32)
        selv[0, 0, c] = 1.0
        in_maps.append({
            "x": xf,
            "xtt": xtt,
            "rwt": rwt,
            "rb": rbv,
            "rbt": rbv.reshape(E, 1).copy(),
            "w1t": w1tile,
            "b1": np.asarray(b1[c]).astype(np.float32).reshape(F, 1),
            "w2t": w2tile,
            "b2": np.asarray(b2[c]).astype(np.float32).reshape(D, 1),
            "sel": selv,
        })
    return in_maps


def combine_results(results):
    out = np.zeros((NTOK, D), np.float32)
    for r in results:
        out += r["out"]
    logits = results[0]["lgout"]
    probs = results[0]["prout"]
    return (out.reshape(2, 2048, D),
            probs.reshape(2, 2048, E),
            logits.reshape(2, 2048, E))


def kernel(x, router_W, router_b, W1, b1, W2, b2):
    from concourse.bass_utils import run_bass_kernel_spmd
    nc = _get_nc()
    in_maps = make_in_maps(x, router_W, router_b, W1, b1, W2, b2)
    res = run_bass_kernel_spmd(nc, in_maps, core_ids=list(range(NCORES)))
    return combine_results(res.results)


# revision 33
# speedup vs baseline: 1.4424x; 1.0171x over previous
"""Expert-parallel top-1 MoE FFN kernel for 8 Trainium2 NeuronCores.

Problem: x[2,2048,1024] routed (top-1 of softmax(x @ rW.T + rb)) through one of
E=8 expert FFNs (Linear(1024,4096) -> gelu -> Linear(4096,1024)).
Returns (expert_outputs, router_probs, router_logits).

Strategy (one expert per core, fully independent cores — no collectives;
an NRT start barrier + AllGather measured ~84us, more than recomputing the
router locally):
  - every core computes the full router (all 4096 tokens) from a
    host-transposed copy of x (xt = x.T, layout prep in the host wrapper):
    64 dense fp32 matmuls with the tiny router weights stationary produce
    logitsT[8, tok]; small PE transposes flip each 128-token tile back to
    [tok, 8] for softmax/argmax (t = p + 128*i layout)
  - compacts its expert's token ids with an on-device prefix-sum
    (tensor_tensor_scan + triangular matmul) and per-column indirect-DMA
    scatters into a DRAM perm table (pad slots -> OOB, skipped)
  - indirect-DMA gather of those <=C=640 token rows, FFN with float32r
    matmuls; W1/W2 are host-retiled so they stream from HBM exactly once in
    512KB/1MB contiguous chunks; indirect-DMA scatter of outputs
  - host sums the 8 disjoint per-core outputs; router outputs from core 0
"""

import numpy as np

import concourse.bass as bass
import concourse.tile as tile
from concourse import bacc, mybir
from concourse.bass import IndirectOffsetOnAxis
from concourse.masks import make_identity, make_upper_triangular

F32 = mybir.dt.float32
F32R = mybir.dt.float32r
I32 = mybir.dt.int32
AX = mybir.AxisListType
OP = mybir.AluOpType
ACT = mybir.ActivationFunctionType

P = 128
NTOK = 4096
D = 1024
F = 4096
E = 8
NCORES = 8
SLICE = NTOK // NCORES          # router tokens per core
C = 640                         # expert token capacity per core
CHUNKS = [(0, 0, 384), (1, 384, 256)]  # (psum bank, slot offset, width);
                                       # each >=256 wide for f32r full rate
NI = NTOK // P                  # 32 tokens per partition (t = p*32 + i)
ND = D // P                     # 8
NF = F // P                     # 32
NST = C // P                    # 5 slot tiles
OOB_PAD = 100000.0              # slot value for tokens not on this expert


def build_nc(act_fn=None):
    nc = bacc.Bacc("TRN2", target_bir_lowering=False, debug=False,
                   num_devices=NCORES)

    x = nc.dram_tensor("x", [NTOK, D], F32, kind="ExternalInput").ap()
    xtt = nc.dram_tensor("xtt", [NTOK // 512, P, ND, 512], F32,
                         kind="ExternalInput").ap()
    rwt = nc.dram_tensor("rwt", [D, E], F32, kind="ExternalInput").ap()
    rb = nc.dram_tensor("rb", [1, E], F32, kind="ExternalInput").ap()
    rbt = nc.dram_tensor("rbt", [E, 1], F32, kind="ExternalInput").ap()
    w1t = nc.dram_tensor("w1t", [NF, P, ND, P], F32, kind="ExternalInput").ap()
    b1 = nc.dram_tensor("b1", [F, 1], F32, kind="ExternalInput").ap()
    w2t = nc.dram_tensor("w2t", [ND, P, NF, P], F32, kind="ExternalInput").ap()
    b2 = nc.dram_tensor("b2", [D, 1], F32, kind="ExternalInput").ap()
    sel = nc.dram_tensor("sel", [1, 1, E], F32, kind="ExternalInput").ap()

    out = nc.dram_tensor("out", [NTOK, D], F32, kind="ExternalOutput").ap()
    lgout = nc.dram_tensor("lgout", [NTOK, E], F32, kind="ExternalOutput").ap()
    prout = nc.dram_tensor("prout", [NTOK, E], F32, kind="ExternalOutput").ap()

    with tile.TileContext(nc) as tc:
        build_kernel(tc, x, xtt, rwt, rb, rbt, w1t, b1, w2t, b2, sel,
                     out, lgout, prout, act_fn=act_fn or ACT.Gelu)
    nc.compile()
    return nc


def build_kernel(tc, x, xtt, rwt, rb, rbt, w1t, b1, w2t, b2, sel,
                 out, lgout, prout, act_fn=ACT.Gelu):
    nc = tc.nc
    with (
        tc.tile_pool(name="const", bufs=1) as const,
        tc.tile_pool(name="small", bufs=2) as small,
        tc.tile_pool(name="routr", bufs=4) as routr,
        tc.tile_pool(name="xg", bufs=1) as xg_pool,
        tc.tile_pool(name="hT", bufs=1) as hT_pool,
        tc.tile_pool(name="psA", bufs=2, space="PSUM") as psA,      # 2x2 banks
        tc.tile_pool(name="psB", bufs=1, space="PSUM") as psB,      # 1x2 banks
        tc.tile_pool(name="psT", bufs=2, space="PSUM") as psT,      # 2x1 banks
        tc.tile_pool(name="dram", bufs=1, space="DRAM") as dram,
    ):
        # ---- constants ----
        ident = const.tile([P, P], F32)
        make_identity(nc, ident[:])
        ut = const.tile([P, P], F32)                  # strictly-upper ones
        make_upper_triangular(nc, ut[:], val=1.0, diag=False)
        rwt_sb = const.tile([P, ND, E], F32)
        nc.sync.dma_start(out=rwt_sb[:], in_=rwt.rearrange("(dt p) e -> p dt e", p=P))
        rb_sb = const.tile([P, E], F32)
        nc.sync.dma_start(out=rb_sb[:], in_=rb.to_broadcast([P, E]))
        rbt_sb = const.tile([E, 1], F32)
        nc.sync.dma_start(out=rbt_sb[:], in_=rbt[:, :])
        sel_sb = const.tile([P, 1, E], F32)
        nc.sync.dma_start(out=sel_sb[:], in_=sel.to_broadcast([P, 1, E]))
        b1_sb = const.tile([P, NF], F32)
        nc.sync.dma_start(out=b1_sb[:], in_=b1.rearrange("(ft p) one -> p (ft one)", p=P))
        b2_sb = const.tile([P, ND], F32)
        nc.sync.dma_start(out=b2_sb[:], in_=b2.rearrange("(dt p) one -> p (dt one)", p=P))
        zeros32 = const.tile([P, NI], F32)
        nc.vector.memset(zeros32[:], 0.0)
        jrow = const.tile([P, C], F32)

        with (
            tc.tile_pool(name="xtp", bufs=3) as xt_pool,
            tc.tile_pool(name="xgp", bufs=2) as xgrow_pool,
        ):
            # ---- phase 1: full router on every core (t = p + 128*i) ----
            # logitsT[8, tok] = rwt.T @ xt, router weights stationary.
            jrow_i = small.tile([P, C], I32, tag="mi", bufs=3)
            nc.gpsimd.iota(jrow_i[:], pattern=[[1, C]], base=0,
                           channel_multiplier=0)
            nc.vector.tensor_copy(out=jrow[:], in_=jrow_i[:])
            mask = small.tile([P, NI], F32, tag="mask")
            TCH = 512        # tokens per router matmul chunk
            for tch in range(NTOK // TCH):
                xt_sb = xt_pool.tile([P, ND, TCH], F32, tag="xt")
                nc.sync.dma_start(out=xt_sb[:], in_=xtt[tch])
                pslt = psT.tile([E, TCH], F32, space="PSUM", tag="pst")
                for dt_ in range(ND):
                    nc.tensor.matmul(pslt[:], lhsT=rwt_sb[:, dt_, :],
                                     rhs=xt_sb[:, dt_, :],
                                     start=(dt_ == 0), stop=(dt_ == ND - 1))
                lgT = routr.tile([E, TCH], F32, tag="lgT")
                nc.vector.tensor_scalar_add(lgT[:], pslt[:], rbt_sb[:, :])
                lgc = routr.tile([P, TCH // P, E], F32, tag="lgc")
                prc = routr.tile([P, TCH // P, E], F32, tag="prc")
                for j in range(TCH // P):
                    i = tch * (TCH // P) + j
                    pslg = psT.tile([P, E], F32, space="PSUM", tag="pst")
                    nc.tensor.transpose(out=pslg[:],
                                        in_=lgT[:, j * P:(j + 1) * P],
                                        identity=ident[:E, :E])
                    lg = lgc[:, j, :]
                    nc.vector.tensor_copy(out=lg, in_=pslg[:])
                    # own-expert mask column (critical path)
                    mx = routr.tile([P, 1], F32, tag="mx")
                    nc.vector.tensor_reduce(out=mx[:], in_=lg, axis=AX.X,
                                            op=OP.max)
                    oh = routr.tile([P, E], F32, tag="oh")
                    nc.vector.tensor_tensor(out=oh[:], in0=lg,
                                            in1=mx[:].to_broadcast([P, E]),
                                            op=OP.is_equal)
                    ohs = routr.tile([P, E], F32, tag="ohs")
                    nc.vector.tensor_tensor(out=ohs[:], in0=oh[:],
                                            in1=sel_sb[:, 0, :], op=OP.mult)
                    nc.vector.tensor_reduce(out=mask[:, i:i + 1], in_=ohs[:],
                                            axis=AX.X, op=OP.add)
                    # softmax (off critical path)
                    negmx = routr.tile([P, 1], F32, tag="negmx")
                    nc.vector.tensor_reduce(out=negmx[:], in_=lg, axis=AX.X,
                                            op=OP.max, negate=True)
                    ex = routr.tile([P, E], F32, tag="ex")
                    nc.scalar.activation(out=ex[:], in_=lg, func=ACT.Exp,
                                         bias=negmx[:, :])
                    sm = routr.tile([P, 1], F32, tag="sm")
                    nc.vector.tensor_reduce(out=sm[:], in_=ex[:], axis=AX.X,
                                            op=OP.add)
                    rcp = routr.tile([P, 1], F32, tag="rcp")
                    nc.vector.reciprocal(out=rcp[:], in_=sm[:])
                    nc.vector.tensor_scalar_mul(prc[:, j, :], ex[:], rcp[:, :])
                nc.scalar.dma_start(
                    out=lgout[tch * TCH:(tch + 1) * TCH, :]
                        .rearrange("(j p) e -> p j e", p=P),
                    in_=lgc[:])
                nc.scalar.dma_start(
                    out=prout[tch * TCH:(tch + 1) * TCH, :]
                        .rearrange("(j p) e -> p j e", p=P),
                    in_=prc[:])

            # ---- phase 3: compaction (matmul-based, no DRAM roundtrip) ----
            cum = small.tile([P, NI], F32, tag="cum")
            nc.vector.tensor_tensor_scan(out=cum[:], data0=mask[:], data1=zeros32[:],
                                         initial=0.0, op0=OP.add, op1=OP.max)
            pspp = psT.tile([P, 1], F32, space="PSUM", tag="pst")
            nc.tensor.matmul(pspp[:], lhsT=ut[:], rhs=cum[:, NI - 1:NI],
                             start=True, stop=True)
            pp = small.tile([P, 1], F32, tag="pp")
            nc.vector.tensor_copy(out=pp[:], in_=pspp[:])
            slot = small.tile([P, NI], F32, tag="slot")
            nc.vector.tensor_tensor(out=slot[:], in0=cum[:], in1=mask[:],
                                    op=OP.subtract)
            nc.vector.tensor_tensor(out=slot[:], in0=slot[:],
                                    in1=pp[:].to_broadcast([P, NI]), op=OP.add)
            # slots for masked-out tokens -> huge (matches no j, so dropped)
            nc.vector.tensor_scalar_add(slot[:], slot[:], -OOB_PAD)
            nc.vector.tensor_tensor(out=slot[:], in0=slot[:], in1=mask[:],
                                    op=OP.mult)
            nc.vector.tensor_scalar_add(slot[:], slot[:], OOB_PAD)
            # rhs pairs [tokid, 1.0] per token column
            tokid = small.tile([P, NI], I32, tag="tokid")
            nc.gpsimd.iota(tokid[:], pattern=[[P, NI]], base=0,
                           channel_multiplier=1)
            tok2 = small.tile([P, NI, 2], F32, tag="tok2")
            nc.vector.memset(tok2[:], 1.0)
            nc.vector.tensor_copy(out=tok2[:, :, 0], in_=tokid[:])
            # gidx[j] = sum_t tokid[t]*[slot(t)==j]; coverage in column 1.
            # Batched 4 token-columns per DVE op to amortize op overheads.
            NB = 4
            gsum4 = small.tile([P, NB, NST, 2], F32, tag="gsum4")
            nc.vector.memset(gsum4[:], 0.0)
            for g in range(NI // NB):
                mi4 = small.tile([P, NB, C], F32, tag="mi4", bufs=2)
                slot4 = slot[:, g * NB:(g + 1) * NB]
                slot4b = bass.AP(tensor=slot4.tensor, offset=slot4.offset,
                                 ap=[*slot4.ap, [0, C]])
                jrow4 = bass.AP(tensor=jrow[:].tensor, offset=jrow[:].offset,
                                ap=[jrow[:].ap[0], [0, NB], jrow[:].ap[1]])
                nc.vector.tensor_tensor(out=mi4[:], in0=jrow4, in1=slot4b,
                                        op=OP.is_equal)
                psg = psT.tile([P, NB, NST, 2], F32, space="PSUM", tag="pst")
                for sub in range(NB):
                    for st in range(NST):
                        nc.tensor.matmul(psg[:, sub, st, :],
                                         lhsT=mi4[:, sub, st * P:(st + 1) * P],
                                         rhs=tok2[:, g * NB + sub, :],
                                         start=True, stop=True)
                nc.vector.tensor_tensor(out=gsum4[:], in0=gsum4[:], in1=psg[:],
                                        op=OP.add)
            gsum = small.tile([P, NST, 2], F32, tag="gsum")
            nc.vector.tensor_tensor(out=gsum[:], in0=gsum4[:, 0], in1=gsum4[:, 1],
                                    op=OP.add)
            nc.vector.tensor_tensor(out=gsum[:], in0=gsum[:], in1=gsum4[:, 2],
                                    op=OP.add)
            nc.vector.tensor_tensor(out=gsum[:], in0=gsum[:], in1=gsum4[:, 3],
                                    op=OP.add)
            # pad slots (coverage 0) -> NTOK (gather clamps; scatter skips)
            gf = small.tile([P, NST], F32, tag="gf")
            nc.vector.tensor_scalar(out=gf[:], in0=gsum[:, :, 1], scalar1=-float(NTOK),
                                    scalar2=float(NTOK), op0=OP.mult, op1=OP.add)
            nc.vector.tensor_tensor(out=gf[:], in0=gf[:], in1=gsum[:, :, 0],
                                    op=OP.add)
            gidx = small.tile([P, NST], I32, tag="gidx")
            nc.vector.tensor_copy(out=gidx[:], in_=gf[:])
            nc.vector.tensor_scalar_min(gf[:], gf[:], float(NTOK - 1))
            gci = small.tile([P, NST], I32, tag="gci")
            nc.vector.tensor_copy(out=gci[:], in_=gf[:])

            # ---- phase 4: gather x rows, transpose to [d, slot] ----
            xgT = xg_pool.tile([P, ND, C], F32R)
            for st in range(NST):
                xg = xgrow_pool.tile([P, D], F32, tag="xg")
                nc.gpsimd.indirect_dma_start(
                    out=xg[:], out_offset=None,
                    in_=x[:, :],
                    in_offset=IndirectOffsetOnAxis(ap=gci[:, st:st + 1], axis=0),
                )
                for dt_ in range(ND):
                    pst = psT.tile([P, P], F32, space="PSUM", tag="pst")
                    nc.tensor.transpose(out=pst[:],
                                        in_=xg[:, dt_ * P:(dt_ + 1) * P],
                                        identity=ident[:])
                    nc.vector.tensor_copy(out=xgT[:, dt_, st * P:(st + 1) * P],
                                          in_=pst[:])

        # ---- phase 5: layer 1 (hT[f, slot] = gelu(W1.T @ xgT + b1)) ----
        ffn_pools = tc.tile_pool(name="w1p", bufs=2)
        w1p = ffn_pools.__enter__()
        w2p_cm = tc.tile_pool(name="w2p", bufs=2)
        w2p = w2p_cm.__enter__()
        yy_cm = tc.tile_pool(name="yy", bufs=1)
        y_pool = yy_cm.__enter__()
        hT = hT_pool.tile([P, NF, C], F32R)
        for fpair in range(NF // 2):
            w1row = w1p.tile([P, 2, ND, P], F32R, tag="w1")
            nc.sync.dma_start(
                out=w1row[:],
                in_=w1t[fpair * 2:(fpair + 1) * 2].bitcast(F32R)
                    .rearrange("f p dt j -> p f dt j"))
            for sub in range(2):
                ft = fpair * 2 + sub
                psh = psA.tile([P, 2, 512], F32, space="PSUM")
                for dt_ in range(ND):
                    for (cb, c0, cn) in CHUNKS:
                        nc.tensor.matmul(
                            psh[:, cb, 0:cn],
                            lhsT=w1row[:, sub, dt_, :],
                            rhs=xgT[:, dt_, c0:c0 + cn],
                            start=(dt_ == 0), stop=(dt_ == ND - 1),
                        )
                for (cb, c0, cn) in CHUNKS:
                    nc.scalar.activation(out=hT[:, ft, c0:c0 + cn],
                                         in_=psh[:, cb, 0:cn],
                                         func=act_fn, bias=b1_sb[:, ft:ft + 1])

        # ---- phase 6: layer 2 (yT[d, slot] = W2.T @ hT + b2), transpose, scatter
        y_tiles = [y_pool.tile([P, D], F32, tag=f"y{st}", name=f"y{st}")
                   for st in range(NST)]
        NFH = NF // 2
        for dt_ in range(ND):
            psy = psB.tile([P, 2, 512], F32, space="PSUM")
            for half in range(2):
                w2row = w2p.tile([P, NFH, P], F32R, tag="w2")
                nc.sync.dma_start(
                    out=w2row[:],
                    in_=w2t[dt_, :, half * NFH:(half + 1) * NFH, :].bitcast(F32R))
                for fh in range(NFH):
                    ft = half * NFH + fh
                    for (cb, c0, cn) in CHUNKS:
                        nc.tensor.matmul(
                            psy[:, cb, 0:cn],
                            lhsT=w2row[:, fh, :],
                            rhs=hT[:, ft, c0:c0 + cn],
                            start=(ft == 0), stop=(ft == NF - 1),
                        )
            yT = small.tile([P, C], F32, tag="yT")
            for (cb, c0, cn) in CHUNKS:
                nc.vector.tensor_scalar_add(yT[:, c0:c0 + cn], psy[:, cb, 0:cn],
                                            b2_sb[:, dt_:dt_ + 1])
            for st in range(NST):
                pst = psT.tile([P, P], F32, space="PSUM", tag="pst")
                nc.tensor.transpose(out=pst[:], in_=yT[:, st * P:(st + 1) * P],
                                    identity=ident[:])
                nc.vector.tensor_copy(out=y_tiles[st][:, dt_ * P:(dt_ + 1) * P],
                                      in_=pst[:])
        for st in range(NST):
            nc.gpsimd.indirect_dma_start(
                out=out[:, :],
                out_offset=IndirectOffsetOnAxis(ap=gidx[:, st:st + 1], axis=0),
                in_=y_tiles[st][:],
                in_offset=None,
                bounds_check=NTOK - 1, oob_is_err=False,
            )
        yy_cm.__exit__(None, None, None)
        w2p_cm.__exit__(None, None, None)
        ffn_pools.__exit__(None, None, None)


_NC_CACHE = None


def _get_nc():
    global _NC_CACHE
    if _NC_CACHE is None:
        _NC_CACHE = build_nc()
    return _NC_CACHE


def make_in_maps(x, router_W, router_b, W1, b1, W2, b2):
    xf = np.ascontiguousarray(np.asarray(x).reshape(NTOK, D).astype(np.float32))
    # xtt[tch, p, dt, t] = x[tch*512 + t, dt*128 + p]
    xtt = np.ascontiguousarray(
        xf.reshape(NTOK // 512, 512, ND, P).transpose(0, 3, 2, 1))
    rwt = np.ascontiguousarray(np.asarray(router_W).astype(np.float32).T)  # [D, E]
    rbv = np.asarray(router_b).astype(np.float32).reshape(1, E)
    in_maps = []
    for c in range(NCORES):
        w1c = np.asarray(W1[c]).astype(np.float32)                  # [D, F]
        w2c = np.asarray(W2[c]).astype(np.float32)                  # [F, D]
        # w1t[ft, p, dt, j] = W1[dt*128+p, ft*128+j]
        w1tile = np.ascontiguousarray(
            w1c.reshape(ND, P, NF, P).transpose(2, 1, 0, 3))
        # w2t[dt, p, ft, j] = W2[ft*128+p, dt*128+j]
        w2tile = np.ascontiguousarray(
            w2c.reshape(NF, P, ND, P).transpose(2, 1, 0, 3))
        selv = np.zeros((1, 1, E), np.float32)
        selv[0, 0, c] = 1.0
        in_maps.append({
            "x": xf,
            "xtt": xtt,
            "rwt": rwt,
            "rb": rbv,
            "rbt": rbv.reshape(E, 1).copy(),
            "w1t": w1tile,
            "b1": np.asarray(b1[c]).astype(np.float32).reshape(F, 1),
            "w2t": w2tile,
            "b2": np.asarray(b2[c]).astype(np.float32).reshape(D, 1),
            "sel": selv,
        })
    return in_maps


def combine_results(results):
    out = np.zeros((NTOK, D), np.float32)
    for r in results:
        out += r["out"]
    logits = results[0]["lgout"]
    probs = results[0]["prout"]
    return (out.reshape(2, 2048, D),
            probs.reshape(2, 2048, E),
            logits.reshape(2, 2048, E))


def kernel(x, router_W, router_b, W1, b1, W2, b2):
    from concourse.bass_utils import run_bass_kernel_spmd
    nc = _get_nc()
    in_maps = make_in_maps(x, router_W, router_b, W1, b1, W2, b2)
    res = run_bass_kernel_spmd(nc, in_maps, core_ids=list(range(NCORES)))
    return combine_results(res.results)
